# revision 2
# baseline (speedup 1.0000x reference)
import math
from contextlib import ExitStack

import numpy as np

P = 128
HID = 128
HEADS = 4
DH = 32
L = 2
EPS = 1e-5
NCORES = 8
MACRO = 4


def _prep(inputs):
    import ml_dtypes

    x = np.ascontiguousarray(np.asarray(inputs["x"], dtype=np.float32))
    rw = np.ascontiguousarray(np.asarray(inputs["rw_diag"], dtype=np.float32))
    ei = np.asarray(inputs["edge_index"]).astype(np.int64)
    psrc = np.asarray(inputs["src"]).astype(np.int64)
    pdst = np.asarray(inputs["dst"]).astype(np.int64)

    N = x.shape[0]
    NT = math.ceil(N / (NCORES * P))
    NLOC = NT * P
    NPADT = NLOC * NCORES

    W_rwse = np.asarray(inputs["W_rwse"], np.float32)
    b_rwse = np.asarray(inputs["b_rwse"], np.float32)
    W_in = np.asarray(inputs["W_in"], np.float32)
    b_in = np.asarray(inputs["b_in"], np.float32)
    IN_C = x.shape[1]
    W_in1 = np.ascontiguousarray(W_in[:IN_C])
    W2 = np.ascontiguousarray(W_rwse @ W_in[IN_C:])
    bcat = b_in + b_rwse @ W_in[IN_C:]

    Wq = np.asarray(inputs["Wq"], np.float32)
    Wk = np.asarray(inputs["Wk"], np.float32)
    Wv = np.asarray(inputs["Wv"], np.float32)
    Ws = np.asarray(inputs["Ws"], np.float32)
    bq = np.asarray(inputs["bq"], np.float32)
    bk = np.asarray(inputs["bk"], np.float32)
    bv = np.asarray(inputs["bv"], np.float32)
    bs = np.asarray(inputs["bs"], np.float32)
    ln_g = np.asarray(inputs["ln_g"], np.float32)
    ln_b = np.asarray(inputs["ln_b"], np.float32)

    Wcat = [np.ascontiguousarray(np.concatenate(
        [Wq[l], Wk[l], Wv[l], Ws[l]], axis=1)) for l in range(L)]
    bqkvs = [np.concatenate([bq[l], bk[l], bv[l], bs[l]]) for l in range(L)]

    row = ei[0]
    col = ei[1]
    core_of = col // NLOC
    tile_of = (col % NLOC) // P

    order = np.lexsort((row, tile_of, core_of))
    srow = row[order]
    scol = col[order]
    score_of = core_of[order]
    stile_of = tile_of[order]

    flat = score_of * NT + stile_of
    cnt = np.bincount(flat, minlength=NCORES * NT).reshape(NCORES, NT)
    tcnt = np.maximum(1, np.ceil(cnt / P).astype(np.int64)).max(axis=0)
    total_t = int(tcnt.sum())
    pad_t = (-total_t) % MACRO
    tcnt[-1] += pad_t
    ET = int(tcnt.sum())
    EPAD = ET * P
    NMACRO = ET // MACRO

    src_idx = np.zeros((NCORES, EPAD), np.int32)
    col_loc = np.zeros((NCORES, EPAD), np.int32)
    oh = np.zeros((NCORES, EPAD, P), ml_dtypes.bfloat16)
    starts = np.concatenate([[0], np.cumsum(tcnt)])[:-1] * P
    gstart = np.concatenate([[0], np.cumsum(cnt.reshape(-1))])
    for c in range(NCORES):
        for t in range(NT):
            g0 = gstart[c * NT + t]
            n = cnt[c, t]
            o = starts[t]
            src_idx[c, o:o + n] = srow[g0:g0 + n]
            cl = (scol[g0:g0 + n] - c * NLOC).astype(np.int32)
            col_loc[c, o:o + n] = cl
            oh[c, np.arange(o, o + n), cl % P] = 1

    def to_hw(a, inner):
        a = a.reshape(NCORES, NMACRO, MACRO, P, inner)
        return np.ascontiguousarray(a.transpose(0, 1, 3, 2, 4))

    src_hw = to_hw(src_idx[..., None], 1)
    col_hw = to_hw(col_loc[..., None], 1)
    oh_hw = to_hw(oh, P)

    sub_tile = np.repeat(np.arange(NT), tcnt)
    sub_start = np.zeros(ET, bool)
    sub_stop = np.zeros(ET, bool)
    sub_start[starts // P] = True
    sub_stop[(starts // P + tcnt - 1)] = True

    NPAIR = psrc.shape[0]
    PLOC = NPAIR // NCORES
    PMAC = math.ceil(PLOC / (4 * P))
    PPAD = PMAC * 4 * P
    ps = np.zeros((NCORES, PPAD), np.int32)
    pd = np.zeros((NCORES, PPAD), np.int32)
    for c in range(NCORES):
        ps[c, :PLOC] = psrc[c * PLOC:(c + 1) * PLOC]
        pd[c, :PLOC] = pdst[c * PLOC:(c + 1) * PLOC]
    ps_hw = to_hw_p = ps.reshape(NCORES, PMAC, 4, P).transpose(0, 1, 3, 2)
    pd_hw = pd.reshape(NCORES, PMAC, 4, P).transpose(0, 1, 3, 2)
    ps_hw = np.ascontiguousarray(ps_hw)
    pd_hw = np.ascontiguousarray(pd_hw)

    xs = np.zeros((NCORES, NLOC, IN_C), np.float32)
    rws = np.zeros((NCORES, NLOC, rw.shape[1]), np.float32)
    for c in range(NCORES):
        lo = c * NLOC
        hi = min(N, lo + NLOC)
        if hi > lo:
            xs[c, :hi - lo] = x[lo:hi]
            rws[c, :hi - lo] = rw[lo:hi]

    return dict(
        N=N, NT=NT, NLOC=NLOC, NPADT=NPADT, ET=ET, EPAD=EPAD, NMACRO=NMACRO,
        NPAIR=NPAIR, PLOC=PLOC, PMAC=PMAC, PPAD=PPAD, IN_C=IN_C,
        W_in1=W_in1, W2=W2, bcat=bcat, Wcat=Wcat, bqkvs=bqkvs,
        ln_g=ln_g, ln_b=ln_b,
        xs=xs, rws=rws, src_hw=src_hw, col_hw=col_hw, oh_hw=oh_hw,
        ps_hw=ps_hw, pd_hw=pd_hw,
        sub_tile=sub_tile, sub_start=sub_start, sub_stop=sub_stop,
        tcnt=tcnt,
    )


def _build(pr):
    import concourse.bass as bass
    import concourse.bacc as bacc
    import concourse.mybir as mybir
    import concourse.tile as tile
    from concourse.masks import make_identity

    f32 = mybir.dt.float32
    bf16 = mybir.dt.bfloat16
    i32 = mybir.dt.int32
    ALU = mybir.AluOpType
    ACT = mybir.ActivationFunctionType

    NT, NLOC, NPADT = pr["NT"], pr["NLOC"], pr["NPADT"]
    NMACRO, PMAC = pr["NMACRO"], pr["PMAC"]
    IN_C = pr["IN_C"]
    scale = 1.0 / math.sqrt(DH)
    bias_nz = bool(np.any(pr["bcat"] != 0))
    qkvs_nz = [bool(np.any(b != 0)) for b in pr["bqkvs"]]
    g_one = [bool(np.all(pr["ln_g"][l] == 1)) for l in range(L)]
    b_zero = [bool(np.all(pr["ln_b"][l] == 0)) for l in range(L)]

    nc = bacc.Bacc(None, num_devices=NCORES)

    t_x = nc.dram_tensor("x_sl", [NLOC, IN_C], f32, kind="ExternalInput")
    t_rw = nc.dram_tensor("rw_sl", [NLOC, 16], f32, kind="ExternalInput")
    t_Win1 = nc.dram_tensor("w_in1", [IN_C, HID], f32, kind="ExternalInput")
    t_W2 = nc.dram_tensor("w2", [16, HID], f32, kind="ExternalInput")
    t_Wc = [nc.dram_tensor(f"wcat{l}", [HID, 4 * HID], f32,
                           kind="ExternalInput") for l in range(L)]
    t_src = nc.dram_tensor("srcidx", [NMACRO, P, MACRO, 1], i32,
                           kind="ExternalInput")
    t_col = nc.dram_tensor("colidx", [NMACRO, P, MACRO, 1], i32,
                           kind="ExternalInput")
    t_oh = nc.dram_tensor("onehot", [NMACRO, P, MACRO, P], bf16,
                          kind="ExternalInput")
    t_ps = nc.dram_tensor("pairsrc", [PMAC, P, 4], i32, kind="ExternalInput")
    t_pd = nc.dram_tensor("pairdst", [PMAC, P, 4], i32, kind="ExternalInput")

    t_h = nc.dram_tensor("h_loc", [NLOC, HID], f32, kind="Internal")
    t_sh = nc.dram_tensor("sh_loc", [NLOC, HID], f32, kind="Internal")
    t_q = nc.dram_tensor("q_loc", [NLOC, HID], bf16, kind="Internal")
    t_kvin = nc.dram_tensor("kv_in", [NLOC, 2 * HID], bf16, kind="Internal")
    t_kv = nc.dram_tensor("kv_full", [NPADT, 2 * HID], bf16, kind="Internal",
                          addr_space="Shared")
    t_hfin = nc.dram_tensor("hf_in", [NLOC, HID], f32, kind="Internal")
    t_hf = nc.dram_tensor("hf_full", [NPADT, HID], f32, kind="Internal",
                          addr_space="Shared")
    t_out = nc.dram_tensor("out", [pr["PPAD"], 1], f32, kind="ExternalOutput")

    rg = [list(range(NCORES))]

    with ExitStack() as ctx:
        tc = ctx.enter_context(tile.TileContext(nc))
        cpool = ctx.enter_context(tc.tile_pool(name="const", bufs=1))
        pa = ctx.enter_context(tc.tile_pool(name="pa", bufs=3))
        ep = ctx.enter_context(tc.tile_pool(name="ep", bufs=4))
        epi = ctx.enter_context(tc.tile_pool(name="epi", bufs=2))
        pp_t = ctx.enter_context(tc.tile_pool(name="ppt", bufs=2, space="PSUM"))
        pp_mm = ctx.enter_context(tc.tile_pool(name="ppm", bufs=2, space="PSUM"))
        pp_agg = ctx.enter_context(tc.tile_pool(name="ppa", bufs=3, space="PSUM"))

        ident = cpool.tile([P, P], f32)
        make_identity(nc, ident[:])
        eps_t = cpool.tile([P, 1], f32)
        nc.vector.memset(eps_t[:], EPS)
        w_in1 = cpool.tile([IN_C, HID], f32)
        nc.sync.dma_start(out=w_in1[:], in_=t_Win1[:, :])
        w2 = cpool.tile([16, HID], f32)
        nc.sync.dma_start(out=w2[:], in_=t_W2[:, :])
        wc = []
        for l in range(L):
            w = cpool.tile([HID, 4 * HID], f32, name=f"wc{l}")
            nc.sync.dma_start(out=w[:], in_=t_Wc[l][:, :])
            wc.append(w)

        def linear_phase(l):
            for t in range(NT):
                sl = slice(t * P, (t + 1) * P)
                if l == 0:
                    x_t = pa.tile([P, IN_C], f32, tag="x_t")
                    nc.sync.dma_start(out=x_t[:], in_=t_x[sl, :])
                    rw_t = pa.tile([P, 16], f32, tag="rw_t")
                    nc.sync.dma_start(out=rw_t[:], in_=t_rw[sl, :])
                    xT_ps = pp_t.tile([P, P], f32, tag="tp")
                    nc.tensor.transpose(out=xT_ps[:], in_=x_t[:],
                                        identity=ident[:])
                    xT = pa.tile([P, P], f32, tag="xT")
                    nc.scalar.copy(out=xT[:], in_=xT_ps[:])
                    rwT_ps = pp_t.tile([16, P], f32, tag="tp",
                                       padded_shape=[P, P])
                    nc.tensor.transpose(out=rwT_ps[:], in_=rw_t[:],
                                        identity=ident[:])
                    rwT = pa.tile([16, P], f32, tag="rwT")
                    nc.scalar.copy(out=rwT[:], in_=rwT_ps[:])
                    h_ps = pp_mm.tile([P, 4 * HID], f32, tag="mm")
                    nc.tensor.matmul(out=h_ps[:, :HID], lhsT=xT[:],
                                     rhs=w_in1[:], start=True, stop=False)
                    nc.tensor.matmul(out=h_ps[:, :HID], lhsT=rwT[:],
                                     rhs=w2[:], start=False, stop=True)
                    h_t = pa.tile([P, HID], f32, tag="h_t")
                    if bias_nz:
                        bc = cpool.tile([1, HID], f32, name="bcat")
                        nc.vector.tensor_tensor(
                            out=h_t[:], in0=h_ps[:, :HID],
                            in1=bc[:].to_broadcast([P, HID]), op=ALU.add)
                    else:
                        nc.scalar.copy(out=h_t[:], in_=h_ps[:, :HID])
                    nc.sync.dma_start(out=t_h[sl, :], in_=h_t[:])
                else:
                    h_t = pa.tile([P, HID], f32, tag="h_t")
                    nc.sync.dma_start(out=h_t[:], in_=t_h[sl, :])
                hT_ps = pp_t.tile([P, P], f32, tag="tp")
                nc.tensor.transpose(out=hT_ps[:], in_=h_t[:], identity=ident[:])
                hT = pa.tile([P, P], f32, tag="hT")
                nc.scalar.copy(out=hT[:], in_=hT_ps[:])
                qk_ps = pp_mm.tile([P, 4 * HID], f32, tag="mm")
                nc.tensor.matmul(out=qk_ps[:], lhsT=hT[:], rhs=wc[l][:],
                                 start=True, stop=True)
                qsb = pa.tile([P, HID], bf16, tag="qsb")
                nc.scalar.copy(out=qsb[:], in_=qk_ps[:, 0:HID])
                kvsb = pa.tile([P, 2 * HID], bf16, tag="kvsb")
                nc.scalar.copy(out=kvsb[:], in_=qk_ps[:, HID:3 * HID])
                shsb = pa.tile([P, HID], f32, tag="shsb")
                nc.vector.tensor_tensor(out=shsb[:], in0=qk_ps[:, 3 * HID:],
                                        in1=h_t[:], op=ALU.add)
                nc.sync.dma_start(out=t_q[sl, :], in_=qsb[:])
                nc.sync.dma_start(out=t_kvin[sl, :], in_=kvsb[:])
                nc.sync.dma_start(out=t_sh[sl, :], in_=shsb[:])

        def epilogue(l, t, agg_ps):
            sl = slice(t * P, (t + 1) * P)
            zsb = epi.tile([P, HEADS], f32, tag="zsb")
            nc.vector.tensor_scalar_max(zsb[:], agg_ps[:, HID:HID + HEADS],
                                        1e-30)
            inv = epi.tile([P, HEADS], f32, tag="inv")
            nc.vector.reciprocal(inv[:], zsb[:])
            sh_t = epi.tile([P, HID], f32, tag="sh_t")
            nc.sync.dma_start(out=sh_t[:], in_=t_sh[sl, :])
            y = epi.tile([P, HID], f32, tag="y")
            nc.vector.tensor_tensor(
                out=y[:].rearrange("p (h d) -> p h d", d=DH),
                in0=agg_ps[:, 0:HID].rearrange("p (h d) -> p h d", d=DH),
                in1=inv[:].unsqueeze(-1).to_broadcast([P, HEADS, DH]),
                op=ALU.mult)
            nc.vector.tensor_tensor(out=y[:], in0=y[:], in1=sh_t[:],
                                    op=ALU.add)
            mu = epi.tile([P, 1], f32, tag="mu")
            nc.vector.reduce_sum(mu[:], y[:], axis=mybir.AxisListType.X)
            nc.vector.tensor_scalar_mul(mu[:], mu[:], 1.0 / HID)
            yc = epi.tile([P, HID], f32, tag="yc")
            nc.vector.tensor_scalar_sub(yc[:], y[:], mu[:])
            junk = epi.tile([P, HID], f32, tag="junk")
            ssq = epi.tile([P, 1], f32, tag="ssq")
            nc.vector.scalar_tensor_tensor(
                out=junk[:], in0=yc[:], scalar=1.0, in1=yc[:],
                op0=ALU.bypass, op1=ALU.mult, accum_out=ssq[:])
            sd = epi.tile([P, 1], f32, tag="sd")
            nc.scalar.activation(out=sd[:], in_=ssq[:], func=ACT.Sqrt,
                                 bias=eps_t[:], scale=1.0 / HID)
            isd = epi.tile([P, 1], f32, tag="isd")
            nc.vector.reciprocal(isd[:], sd[:])
            h_o = epi.tile([P, HID], f32, tag="h_o")
            if g_one[l] and b_zero[l]:
                nc.scalar.activation(out=h_o[:], in_=yc[:], func=ACT.Relu,
                                     scale=isd[:])
            else:
                gt = cpool.tile([1, HID], f32, name=f"lng{l}")
                bt = cpool.tile([1, HID], f32, name=f"lnb{l}")
                nc.vector.tensor_scalar_mul(yc[:], yc[:], isd[:])
                nc.vector.tensor_tensor(out=yc[:], in0=yc[:],
                                        in1=gt[:].to_broadcast([P, HID]),
                                        op=ALU.mult)
                nc.vector.tensor_tensor(out=yc[:], in0=yc[:],
                                        in1=bt[:].to_broadcast([P, HID]),
                                        op=ALU.add)
                nc.scalar.activation(out=h_o[:], in_=yc[:], func=ACT.Relu)
            nc.sync.dma_start(out=t_h[sl, :], in_=h_o[:])
            if l == L - 1:
                nc.sync.dma_start(out=t_hfin[sl, :], in_=h_o[:])

        def edge_phase(l):
            sub_tile = pr["sub_tile"]
            sub_start = pr["sub_start"]
            sub_stop = pr["sub_stop"]
            agg_map = {}
            for m in range(NMACRO):
                sidx = ep.tile([P, MACRO], i32, tag="sidx")
                nc.sync.dma_start(out=sidx[:],
                                  in_=t_src[m].rearrange("p m 1 -> p m"))
                cidx = ep.tile([P, MACRO], i32, tag="cidx")
                nc.sync.dma_start(out=cidx[:],
                                  in_=t_col[m].rearrange("p m 1 -> p m"))
                oh = ep.tile([P, MACRO * P], bf16, tag="oh")
                nc.sync.dma_start(
                    out=oh[:], in_=t_oh[m].rearrange("p m n -> p (m n)"))
                kvg = ep.tile([P, MACRO * 2 * HID], bf16, tag="kvg")
                kvg3 = kvg[:].rearrange("p (m f) -> p m f", f=2 * HID)
                qg = ep.tile([P, MACRO * HID], bf16, tag="qg")
                qg3 = qg[:].rearrange("p (m f) -> p m f", f=HID)
                for j in range(MACRO):
                    nc.gpsimd.indirect_dma_start(
                        out=kvg3[:, j, :], out_offset=None, in_=t_kv[:, :],
                        in_offset=bass.IndirectOffsetOnAxis(
                            ap=sidx[:, j:j + 1], axis=0))
                    nc.gpsimd.indirect_dma_start(
                        out=qg3[:, j, :], out_offset=None, in_=t_q[:, :],
                        in_offset=bass.IndirectOffsetOnAxis(
                            ap=cidx[:, j:j + 1], axis=0))
                prod = ep.tile([P, MACRO * HID], bf16, tag="prod")
                nc.vector.tensor_tensor(
                    out=prod[:].rearrange("p (m f) -> p m f", f=HID),
                    in0=kvg3[:, :, 0:HID], in1=qg3[:, :, :], op=ALU.mult)
                sc = ep.tile([P, MACRO * HEADS], f32, tag="sc")
                nc.vector.reduce_sum(
                    sc[:],
                    prod[:].rearrange("p (m h d) -> p (m h) d", m=MACRO,
                                      h=HEADS, d=DH),
                    axis=mybir.AxisListType.X)
                ext = ep.tile([P, MACRO * HEADS], bf16, tag="ext")
                nc.scalar.activation(
                    out=ext[:], in_=sc[:], func=ACT.Exp, scale=scale)
                ext3 = ext[:].rearrange("p (m h) -> p m h", m=MACRO, h=HEADS)
                rhs = ep.tile([P, MACRO * (HID + HEADS)], bf16, tag="rhs")
                rhs3 = rhs[:].rearrange("p (m f) -> p m f", f=HID + HEADS)
                nc.vector.tensor_copy(rhs3[:, :, HID:HID + HEADS], ext3)
                nc.vector.tensor_tensor(
                    out=rhs3[:, :, 0:HID].rearrange("p m (h d) -> p m h d",
                                                    d=DH),
                    in0=kvg3[:, :, HID:2 * HID].rearrange(
                        "p m (h d) -> p m h d", d=DH),
                    in1=ext3.unsqueeze(-1)
                        .to_broadcast([P, MACRO, HEADS, DH]),
                    op=ALU.mult)
                oh3 = oh[:].rearrange("p (m n) -> p m n", n=P)
                for j in range(MACRO):
                    st = m * MACRO + j
                    t = int(sub_tile[st])
                    if sub_start[st]:
                        agg_map[t] = pp_agg.tile([P, HID + HEADS], f32,
                                                 tag="agg", name="aggt")
                    nc.tensor.matmul(
                        out=agg_map[t][:], lhsT=oh3[:, j, :],
                        rhs=rhs3[:, j, :], start=bool(sub_start[st]),
                        stop=bool(sub_stop[st]))
                    if sub_stop[st]:
                        epilogue(l, t, agg_map.pop(t)[:])

        for l in range(L):
            linear_phase(l)
            nc.gpsimd.collective_compute(
                "AllGather", mybir.AluOpType.bypass, replica_groups=rg,
                ins=[t_kvin[:, :]], outs=[t_kv[:, :]])
            edge_phase(l)

        nc.gpsimd.collective_compute(
            "AllGather", mybir.AluOpType.bypass, replica_groups=rg,
            ins=[t_hfin[:, :]], outs=[t_hf[:, :]])

        for m in range(PMAC):
            sidx = ep.tile([P, 4], i32, tag="psidx")
            nc.sync.dma_start(out=sidx[:], in_=t_ps[m])
            didx = ep.tile([P, 4], i32, tag="pdidx")
            nc.sync.dma_start(out=didx[:], in_=t_pd[m])
            hs = ep.tile([P, 4 * HID], f32, tag="hs")
            hs3 = hs[:].rearrange("p (m f) -> p m f", f=HID)
            hd = ep.tile([P, 4 * HID], f32, tag="hd")
            hd3 = hd[:].rearrange("p (m f) -> p m f", f=HID)
            for j in range(4):
                nc.gpsimd.indirect_dma_start(
                    out=hs3[:, j, :], out_offset=None, in_=t_hf[:, :],
                    in_offset=bass.IndirectOffsetOnAxis(
                        ap=sidx[:, j:j + 1], axis=0))
                nc.gpsimd.indirect_dma_start(
                    out=hd3[:, j, :], out_offset=None, in_=t_hf[:, :],
                    in_offset=bass.IndirectOffsetOnAxis(
                        ap=didx[:, j:j + 1], axis=0))
            pp = ep.tile([P, 4 * HID], f32, tag="pp")
            nc.vector.tensor_tensor(out=pp[:], in0=hs[:], in1=hd[:],
                                    op=ALU.mult)
            dots = ep.tile([P, 4], f32, tag="dots")
            nc.vector.reduce_sum(
                dots[:], pp[:].rearrange("p (m f) -> p m f", f=HID),
                axis=mybir.AxisListType.X)
            osb = ep.tile([P, 4], f32, tag="osb")
            nc.scalar.activation(out=osb[:], in_=dots[:], func=ACT.Sigmoid)
            nc.sync.dma_start(
                out=t_out[m * 4 * P:(m + 1) * 4 * P, 0]
                    .rearrange("(m p) -> p m", p=P),
                in_=osb[:])

    nc.finalize()
    return nc


def kernel(**inputs):
    from concourse.bass_utils import run_bass_kernel_spmd

    pr = _prep(inputs)
    nc = _build(pr)

    in_maps = []
    for c in range(NCORES):
        in_maps.append({
            "x_sl": pr["xs"][c],
            "rw_sl": pr["rws"][c],
            "w_in1": pr["W_in1"],
            "w2": pr["W2"],
            "wcat0": pr["Wcat"][0],
            "wcat1": pr["Wcat"][1],
            "srcidx": pr["src_hw"][c],
            "colidx": pr["col_hw"][c],
            "onehot": pr["oh_hw"][c],
            "pairsrc": pr["ps_hw"][c],
            "pairdst": pr["pd_hw"][c],
        })

    import os
    kw = {}
    if os.environ.get("KERNEL_TMPDIR"):
        kw["tmpdir"] = os.environ["KERNEL_TMPDIR"]
    res = run_bass_kernel_spmd(
        nc, in_maps, core_ids=list(range(NCORES)),
        trace=bool(int(os.environ.get("KERNEL_TRACE", "0"))), **kw)
    if res.exec_time_ns is not None:
        print(f"HW exec time: {res.exec_time_ns} ns")

    PLOC = pr["PLOC"]
    out = np.concatenate(
        [res.results[c]["out"][:PLOC, 0] for c in range(NCORES)])
    return out.astype(np.float32)



# revision 3
# speedup vs baseline: 1.8351x; 1.8351x over previous
import math
import os
from contextlib import ExitStack

import numpy as np

P = 128
HID = 128
HEADS = 4
DH = 32
L = 2
EPS = 1e-5
NCORES = 8
KB = 8
QS = 4
GT = 8
KBP = 8


def _groups(nt, g):
    out = []
    t0 = 0
    while t0 < nt:
        out.append((t0, min(g, nt - t0)))
        t0 += min(g, nt - t0)
    return out


def _prep(inputs):
    import ml_dtypes

    bf = ml_dtypes.bfloat16
    f8 = ml_dtypes.float8_e4m3

    x = np.ascontiguousarray(np.asarray(inputs["x"], dtype=np.float32))
    rw = np.ascontiguousarray(np.asarray(inputs["rw_diag"], dtype=np.float32))
    ei = np.asarray(inputs["edge_index"]).astype(np.int64)
    psrc = np.asarray(inputs["src"]).astype(np.int64)
    pdst = np.asarray(inputs["dst"]).astype(np.int64)

    N = x.shape[0]
    IN_C = x.shape[1]
    RWD = rw.shape[1]
    NT = math.ceil(N / (NCORES * P))
    NLOC = NT * P
    NPADT = NLOC * NCORES

    W_rwse = np.asarray(inputs["W_rwse"], np.float32)
    b_rwse = np.asarray(inputs["b_rwse"], np.float32)
    W_in = np.asarray(inputs["W_in"], np.float32)
    b_in = np.asarray(inputs["b_in"], np.float32)
    W1 = np.ascontiguousarray(W_in[:IN_C]).astype(bf)
    W2 = np.ascontiguousarray(W_rwse @ W_in[IN_C:]).astype(bf)
    bcat = (b_in + b_rwse @ W_in[IN_C:]).astype(np.float32)

    Wq = np.asarray(inputs["Wq"], np.float32)
    Wk = np.asarray(inputs["Wk"], np.float32)
    Wv = np.asarray(inputs["Wv"], np.float32)
    Ws = np.asarray(inputs["Ws"], np.float32)
    bq = np.asarray(inputs["bq"], np.float32)
    bk = np.asarray(inputs["bk"], np.float32)
    bv = np.asarray(inputs["bv"], np.float32)
    bs = np.asarray(inputs["bs"], np.float32)
    ln_g = np.asarray(inputs["ln_g"], np.float32)
    ln_b = np.asarray(inputs["ln_b"], np.float32)

    Wcat = [np.ascontiguousarray(np.concatenate(
        [Wq[l], Wk[l], Wv[l], Ws[l]], axis=1)).astype(bf) for l in range(L)]
    bqkvs = [np.concatenate([bq[l], bk[l], bv[l], bs[l]]) for l in range(L)]

    row = ei[0]
    col = ei[1]
    core_of = col // NLOC
    tile_of = (col % NLOC) // P

    order = np.lexsort((row, tile_of, core_of))
    srow = row[order]
    scol = col[order]

    flat = core_of[order] * NT + tile_of[order]
    cnt = np.bincount(flat, minlength=NCORES * NT).reshape(NCORES, NT)
    tcnt = np.maximum(1, np.ceil(cnt / P).astype(np.int64)).max(axis=0)
    total_t = int(tcnt.sum())
    tcnt[-1] += (-total_t) % KB
    ET = int(tcnt.sum())
    NB = ET // KB

    sub_tile = np.repeat(np.arange(NT), tcnt)
    starts = np.concatenate([[0], np.cumsum(tcnt)])[:-1] * P
    sub_start = np.zeros(ET, bool)
    sub_stop = np.zeros(ET, bool)
    sub_start[starts // P] = True
    sub_stop[(starts // P + tcnt - 1)] = True

    kvidx = np.zeros((NCORES, ET * P), np.int32)
    ohm = np.zeros((NCORES, ET * P, P), bf)
    ohtm = np.zeros((NCORES, ET, P, P), bf)
    gstart = np.concatenate([[0], np.cumsum(cnt.reshape(-1))])
    for c in range(NCORES):
        for t in range(NT):
            g0 = gstart[c * NT + t]
            n = cnt[c, t]
            if n == 0:
                continue
            o = starts[t]
            sl = np.arange(o, o + n)
            kvidx[c, sl] = srow[g0:g0 + n]
            cl = (scol[g0:g0 + n] - c * NLOC).astype(np.int32)
            ohm[c, sl, cl % P] = 1
            ohtm[c, sl // P, cl % P, sl % P] = 1

    kvidx_hw = kvidx.reshape(NCORES, NB, KB, P).transpose(0, 1, 3, 2)
    kvidx_hw = np.ascontiguousarray(kvidx_hw)
    oh_hw = np.ascontiguousarray(
        ohm.reshape(NCORES, NB, KB, P, P).transpose(0, 1, 3, 2, 4)).reshape(
        NCORES, NB, P, KB * P)
    oht_hw = np.ascontiguousarray(
        ohtm.reshape(NCORES, NB, KB, P, P).transpose(0, 1, 3, 2, 4)).reshape(
        NCORES, NB, P, KB * P)

    xs = np.zeros((NCORES, NLOC, IN_C), np.float32)
    rws = np.zeros((NCORES, NLOC, RWD), np.float32)
    for c in range(NCORES):
        lo = c * NLOC
        hi = min(N, lo + NLOC)
        if hi > lo:
            xs[c, :hi - lo] = x[lo:hi]
            rws[c, :hi - lo] = rw[lo:hi]
    xT_hw = np.ascontiguousarray(
        xs.reshape(NCORES, NT, P, IN_C).transpose(0, 1, 3, 2)).astype(bf)
    rwT_hw = np.ascontiguousarray(
        rws.reshape(NCORES, NT, P, RWD).transpose(0, 1, 3, 2)).astype(bf)

    NPAIR = psrc.shape[0]
    PLOC = math.ceil(NPAIR / NCORES)
    plocs = [max(0, min(PLOC, NPAIR - c * PLOC)) for c in range(NCORES)]
    NPS = math.ceil(PLOC / P)
    NPB = math.ceil(NPS / KBP)
    PPAD = NPB * KBP * P
    ps = np.zeros((NCORES, PPAD), np.int32)
    pd = np.zeros((NCORES, PPAD), np.int32)
    for c in range(NCORES):
        n = plocs[c]
        ps[c, :n] = psrc[c * PLOC:c * PLOC + n]
        pd[c, :n] = pdst[c * PLOC:c * PLOC + n]
    pidx_hw = np.zeros((NCORES, NPB, P, 2 * KBP), np.int32)
    pidx_hw[:, :, :, 0:KBP] = ps.reshape(
        NCORES, NPB, KBP, P).transpose(0, 1, 3, 2)
    pidx_hw[:, :, :, KBP:] = pd.reshape(
        NCORES, NPB, KBP, P).transpose(0, 1, 3, 2)

    return dict(
        N=N, IN_C=IN_C, RWD=RWD, NT=NT, NLOC=NLOC, NPADT=NPADT,
        ET=ET, NB=NB, NPB=NPB, PLOC=PLOC, plocs=plocs, NPAIR=NPAIR,
        W1=W1, W2=W2, bcat=bcat, Wcat=Wcat, bqkvs=bqkvs,
        ln_g=ln_g, ln_b=ln_b,
        xT=xT_hw, rwT=rwT_hw, kvidx=kvidx_hw, oh=oh_hw, oht=oht_hw,
        pidx=pidx_hw,
        sub_tile=sub_tile, sub_start=sub_start, sub_stop=sub_stop,
    )


def _build(pr):
    import concourse.bass as bass
    import concourse.bacc as bacc
    import concourse.mybir as mybir
    import concourse.tile as tile

    f32 = mybir.dt.float32
    bf16 = mybir.dt.bfloat16
    fp8 = mybir.dt.float8e4
    i32 = mybir.dt.int32
    ALU = mybir.AluOpType
    ACT = mybir.ActivationFunctionType
    X = mybir.AxisListType.X

    NT, NLOC, NPADT = pr["NT"], pr["NLOC"], pr["NPADT"]
    NB, NPB = pr["NB"], pr["NPB"]
    IN_C, RWD = pr["IN_C"], pr["RWD"]
    scale = 1.0 / math.sqrt(DH)

    bias_nz = bool(np.any(pr["bcat"] != 0))
    qkvs_nz = [bool(np.any(b != 0)) for b in pr["bqkvs"]]
    g_one = [bool(np.all(pr["ln_g"][l] == 1)) for l in range(L)]
    b_zero = [bool(np.all(pr["ln_b"][l] == 0)) for l in range(L)]

    nc = bacc.Bacc(None, num_devices=NCORES)

    t_xT = nc.dram_tensor("x_t", [NT, IN_C, P], bf16, kind="ExternalInput")
    t_rwT = nc.dram_tensor("rw_t", [NT, RWD, P], bf16, kind="ExternalInput")
    t_w1 = nc.dram_tensor("w1", [IN_C, HID], bf16, kind="ExternalInput")
    t_w2 = nc.dram_tensor("w2", [RWD, HID], bf16, kind="ExternalInput")
    t_wc = [nc.dram_tensor(f"wc{l}", [HID, 4 * HID], bf16,
                           kind="ExternalInput") for l in range(L)]
    t_kvi = nc.dram_tensor("kvidx", [NB, P, KB], i32, kind="ExternalInput")
    t_oh = nc.dram_tensor("onehot", [NB, P, KB * P], bf16,
                          kind="ExternalInput")
    t_oht = nc.dram_tensor("onehot_t", [NB, P, KB * P], bf16,
                           kind="ExternalInput")
    t_pidx = nc.dram_tensor("pidx", [NPB, P, 2 * KBP], i32,
                            kind="ExternalInput")

    t_h = nc.dram_tensor("h_loc", [NT, P, HID], bf16, kind="Internal")
    t_kvin = nc.dram_tensor("kv_in", [NLOC, 2 * HID], fp8, kind="Internal")
    t_kv = nc.dram_tensor("kv_full", [NPADT, 2 * HID], fp8,
                          kind="Internal", addr_space="Shared")
    t_hfin = nc.dram_tensor("hf_in", [NLOC, HID], bf16, kind="Internal")
    t_hf = nc.dram_tensor("hf_full", [NPADT, HID], bf16, kind="Internal",
                          addr_space="Shared")
    t_out = nc.dram_tensor("out", [NPB * KBP, P], f32,
                           kind="ExternalOutput")

    rg = [list(range(NCORES))]
    groups = _groups(NT, GT)

    with ExitStack() as ctx:
        tc = ctx.enter_context(tile.TileContext(nc))
        cpool = ctx.enter_context(tc.tile_pool(name="const", bufs=1))
        lp = ctx.enter_context(tc.tile_pool(name="lp", bufs=2))
        ep = ctx.enter_context(tc.tile_pool(name="ep", bufs=3))
        epi = ctx.enter_context(tc.tile_pool(name="epi", bufs=2))
        pp_mm = ctx.enter_context(tc.tile_pool(name="ppm", bufs=3,
                                               space="PSUM"))
        pp_agg = ctx.enter_context(tc.tile_pool(name="ppa", bufs=3,
                                                space="PSUM"))
        pp_qs = ctx.enter_context(tc.tile_pool(name="ppq", bufs=2,
                                               space="PSUM"))

        w1 = cpool.tile([IN_C, HID], bf16)
        nc.sync.dma_start(out=w1[:], in_=t_w1[:, :])
        w2 = cpool.tile([RWD, HID], bf16)
        nc.sync.dma_start(out=w2[:], in_=t_w2[:, :])
        wc = []
        for l in range(L):
            w = cpool.tile([HID, 4 * HID], bf16, name=f"wc{l}")
            nc.sync.dma_start(out=w[:], in_=t_wc[l][:, :])
            wc.append(w)

        sh_all = cpool.tile([P, NT * HID], bf16, name="sh_all")
        yc_all = cpool.tile([P, NT * HID], bf16, name="yc_all")
        q_all = cpool.tile([P, NT * HID], bf16, name="q_all")
        var_all = cpool.tile([P, NT], f32, name="var_all")
        isd_all = cpool.tile([P, NT], f32, name="isd_all")

        def linear_phase(l):
            for (g0, gsz) in groups:
                hT_sl = lp.tile([P, GT * P], bf16, tag="hT")
                h_sl = lp.tile([P, GT * P], bf16, tag="h")
                if l == 0:
                    xT_sl = lp.tile([P, GT * P], bf16, tag="xT")
                    nc.sync.dma_start(
                        out=xT_sl[:, :gsz * P],
                        in_=t_xT[g0:g0 + gsz].rearrange("g c n -> c g n"))
                    rwT_sl = lp.tile([RWD, GT * P], bf16, tag="rwT")
                    nc.sync.dma_start(
                        out=rwT_sl[:, :gsz * P],
                        in_=t_rwT[g0:g0 + gsz].rearrange("g c n -> c g n"))
                    for j in range(gsz):
                        sl = slice(j * P, (j + 1) * P)
                        hT_ps = pp_mm.tile([P, P], f32, tag="mm0")
                        nc.tensor.matmul(out=hT_ps[:], lhsT=w1[:],
                                         rhs=xT_sl[:, sl], start=True,
                                         stop=False)
                        nc.tensor.matmul(out=hT_ps[:], lhsT=w2[:],
                                         rhs=rwT_sl[:, sl], start=False,
                                         stop=True)
                        h_ps = pp_mm.tile([P, P], f32, tag="mm0")
                        nc.tensor.matmul(out=h_ps[:], lhsT=xT_sl[:, sl],
                                         rhs=w1[:], start=True, stop=False)
                        nc.tensor.matmul(out=h_ps[:], lhsT=rwT_sl[:, sl],
                                         rhs=w2[:], start=False, stop=True)
                        nc.scalar.copy(out=hT_sl[:, sl], in_=hT_ps[:])
                        nc.scalar.copy(out=h_sl[:, sl], in_=h_ps[:])
                    if bias_nz:
                        bc_r = cpool.tile([1, HID], f32, name="bc_r")
                        bc_c = cpool.tile([P, 1], f32, name="bc_c")
                        nc.vector.tensor_tensor(
                            out=h_sl[:, :gsz * P].rearrange(
                                "p (g f) -> p g f", f=HID),
                            in0=h_sl[:, :gsz * P].rearrange(
                                "p (g f) -> p g f", f=HID),
                            in1=bc_r[:].to_broadcast([P, gsz, HID]),
                            op=ALU.add)
                        nc.vector.tensor_scalar_add(
                            hT_sl[:, :gsz * P], hT_sl[:, :gsz * P], bc_c[:])
                else:
                    nc.sync.dma_start(
                        out=hT_sl[:, :gsz * P],
                        in_=t_h[g0:g0 + gsz].rearrange("g p f -> (g p) f"),
                        transpose=True)
                    nc.scalar.dma_start(
                        out=h_sl[:, :gsz * P],
                        in_=t_h[g0:g0 + gsz].rearrange("g p f -> p g f"))
                kvs_sl = lp.tile([P, GT * 2 * HID], fp8, tag="kvs")
                for j in range(gsz):
                    t = g0 + j
                    sl = slice(j * P, (j + 1) * P)
                    qk_ps = pp_mm.tile([P, 4 * HID], f32, tag="mm0")
                    nc.tensor.matmul(out=qk_ps[:], lhsT=hT_sl[:, sl],
                                     rhs=wc[l][:], start=True, stop=True)
                    if qkvs_nz[l]:
                        bqk = cpool.tile([1, 4 * HID], f32, name=f"bqk{l}")
                        nc.vector.tensor_tensor(
                            out=qk_ps[:], in0=qk_ps[:],
                            in1=bqk[:].to_broadcast([P, 4 * HID]), op=ALU.add)
                    nc.scalar.copy(out=q_all[:, t * HID:(t + 1) * HID],
                                   in_=qk_ps[:, 0:HID])
                    nc.scalar.copy(
                        out=kvs_sl[:, j * 2 * HID:(j + 1) * 2 * HID],
                        in_=qk_ps[:, HID:3 * HID])
                    nc.vector.tensor_tensor(
                        out=sh_all[:, t * HID:(t + 1) * HID],
                        in0=qk_ps[:, 3 * HID:4 * HID], in1=h_sl[:, sl],
                        op=ALU.add)
                nc.sync.dma_start(
                    out=t_kvin[g0 * P:(g0 + gsz) * P, :].rearrange(
                        "(g p) f -> p g f", p=P),
                    in_=kvs_sl[:, :gsz * 2 * HID])

        def pass1(l, t, agg):
            zsb = epi.tile([P, HEADS], f32, tag="zsb")
            nc.vector.tensor_scalar_max(zsb[:], agg[:, HID:HID + HEADS],
                                        1e-30)
            inv = epi.tile([P, HEADS], f32, tag="inv")
            nc.vector.reciprocal(inv[:], zsb[:])
            y = epi.tile([P, HID], bf16, tag="y")
            nc.vector.tensor_tensor(
                out=y[:].rearrange("p (h d) -> p h d", d=DH),
                in0=agg[:, 0:HID].rearrange("p (h d) -> p h d", d=DH),
                in1=inv[:].unsqueeze(-1).to_broadcast([P, HEADS, DH]),
                op=ALU.mult)
            nc.vector.tensor_tensor(
                out=y[:], in0=y[:], in1=sh_all[:, t * HID:(t + 1) * HID],
                op=ALU.add)
            musum = epi.tile([P, 1], f32, tag="musum")
            nc.vector.reduce_sum(musum[:], y[:], axis=X)
            mu = epi.tile([P, 1], f32, tag="mu")
            nc.vector.tensor_scalar_mul(mu[:], musum[:], 1.0 / HID)
            yc = yc_all[:, t * HID:(t + 1) * HID]
            nc.vector.tensor_scalar_sub(yc, y[:], mu[:])
            junk = epi.tile([P, HID], bf16, tag="junk")
            ssq = epi.tile([P, 1], f32, tag="ssq")
            nc.vector.scalar_tensor_tensor(
                out=junk[:], in0=yc, scalar=1.0, in1=yc,
                op0=ALU.bypass, op1=ALU.mult, accum_out=ssq[:])
            nc.vector.tensor_scalar(
                out=var_all[:, t:t + 1], in0=ssq[:], scalar1=1.0 / HID,
                scalar2=EPS, op0=ALU.mult, op1=ALU.add)

        def edge_phase(l):
            sub_tile = pr["sub_tile"]
            sub_start = pr["sub_start"]
            sub_stop = pr["sub_stop"]
            agg_map = {}
            for b in range(NB):
                idx = ep.tile([P, KB], i32, tag="idx")
                nc.sync.dma_start(out=idx[:], in_=t_kvi[b])
                oh = ep.tile([P, KB * P], bf16, tag="oh")
                nc.sync.dma_start(out=oh[:], in_=t_oh[b])
                oht = ep.tile([P, KB * P], bf16, tag="oht")
                nc.scalar.dma_start(out=oht[:], in_=t_oht[b])
                oht3 = oht[:].rearrange("p (k n) -> p k n", n=P)
                kvg = ep.tile([P, KB * 2 * HID], bf16, tag="kvg")
                kvg3 = kvg[:].rearrange("p (k f) -> p k f", f=2 * HID)
                qg = ep.tile([P, KB * HID], bf16, tag="qg")
                for j in range(KB):
                    nc.gpsimd.indirect_dma_start(
                        out=kvg3[:, j, :], out_offset=None, in_=t_kv[:, :],
                        in_offset=bass.IndirectOffsetOnAxis(
                            ap=idx[:, j:j + 1], axis=0))
                for j0 in range(0, KB, QS):
                    qs_ps = pp_qs.tile([P, QS * HID], f32, tag="qs")
                    for j in range(j0, j0 + QS):
                        st = b * KB + j
                        t = int(sub_tile[st])
                        nc.tensor.matmul(
                            out=qs_ps[:, (j - j0) * HID:(j - j0 + 1) * HID],
                            lhsT=oht3[:, j, :],
                            rhs=q_all[:, t * HID:(t + 1) * HID],
                            start=True, stop=True)
                    nc.scalar.copy(
                        out=qg[:, j0 * HID:(j0 + QS) * HID], in_=qs_ps[:])
                prod = ep.tile([P, KB * HID], bf16, tag="prod")
                nc.vector.tensor_tensor(
                    out=prod[:].rearrange("p (k f) -> p k f", f=HID),
                    in0=kvg3[:, :, 0:HID],
                    in1=qg[:].rearrange("p (k f) -> p k f", f=HID),
                    op=ALU.mult)
                sc = ep.tile([P, KB * HEADS], f32, tag="sc")
                nc.vector.reduce_sum(
                    sc[:], prod[:].rearrange("p (g d) -> p g d", d=DH),
                    axis=X)
                ext = ep.tile([P, KB * HID], bf16, tag="ext")
                nc.scalar.activation(
                    out=ext[:].rearrange("p (k h d) -> p k h d", h=HEADS,
                                         d=DH),
                    in_=sc[:].rearrange("p (k h) -> p k h", h=HEADS)
                        .unsqueeze(-1).to_broadcast([P, KB, HEADS, DH]),
                    func=ACT.Exp, scale=scale)
                rhs = ep.tile([P, KB * (HID + HEADS)], bf16, tag="rhs")
                rhs3 = rhs[:].rearrange("p (k f) -> p k f", f=HID + HEADS)
                nc.scalar.activation(
                    out=rhs3[:, :, HID:HID + HEADS],
                    in_=sc[:].rearrange("p (k h) -> p k h", h=HEADS),
                    func=ACT.Exp, scale=scale)
                nc.vector.tensor_tensor(
                    out=rhs3[:, :, 0:HID], in0=kvg3[:, :, HID:2 * HID],
                    in1=ext[:].rearrange("p (k f) -> p k f", f=HID),
                    op=ALU.mult)
                oh3 = oh[:].rearrange("p (k n) -> p k n", n=P)
                for j in range(KB):
                    st = b * KB + j
                    t = int(sub_tile[st])
                    if sub_start[st]:
                        agg_map[t] = pp_agg.tile([P, HID + HEADS], f32,
                                                 tag="agg", name="aggt")
                    nc.tensor.matmul(
                        out=agg_map[t][:], lhsT=oh3[:, j, :],
                        rhs=rhs3[:, j, :], start=bool(sub_start[st]),
                        stop=bool(sub_stop[st]))
                    if sub_stop[st]:
                        pass1(l, t, agg_map.pop(t)[:])

        def finish_layer(l):
            sd = epi.tile([P, NT], f32, tag="sd")
            nc.scalar.activation(out=sd[:], in_=var_all[:], func=ACT.Sqrt)
            nc.vector.reciprocal(isd_all[:], sd[:])
            gen = not (g_one[l] and b_zero[l])
            if gen:
                lng = cpool.tile([1, HID], f32, name=f"lng{l}")
                lnb = cpool.tile([1, HID], f32, name=f"lnb{l}")
            for (g0, gsz) in groups:
                h_sl = lp.tile([P, GT * HID], bf16, tag="ho")
                for j in range(gsz):
                    t = g0 + j
                    sl = slice(j * HID, (j + 1) * HID)
                    ycb = yc_all[:, t * HID:(t + 1) * HID]
                    if gen:
                        tmp = epi.tile([P, HID], bf16, tag="tmp")
                        nc.vector.tensor_scalar_mul(
                            tmp[:], ycb, isd_all[:, t:t + 1])
                        nc.vector.tensor_tensor(
                            out=tmp[:], in0=tmp[:],
                            in1=lng[:].to_broadcast([P, HID]), op=ALU.mult)
                        nc.vector.tensor_tensor(
                            out=tmp[:], in0=tmp[:],
                            in1=lnb[:].to_broadcast([P, HID]), op=ALU.add)
                        nc.vector.tensor_scalar_max(h_sl[:, sl], tmp[:], 0.0)
                    else:
                        nc.vector.tensor_scalar(
                            out=h_sl[:, sl], in0=ycb,
                            scalar1=isd_all[:, t:t + 1], scalar2=0.0,
                            op0=ALU.mult, op1=ALU.max)
                if l < L - 1:
                    nc.sync.dma_start(
                        out=t_h[g0:g0 + gsz].rearrange("g p f -> p g f"),
                        in_=h_sl[:, :gsz * HID])
                else:
                    nc.sync.dma_start(
                        out=t_hfin[g0 * P:(g0 + gsz) * P, :].rearrange(
                            "(g p) f -> p g f", p=P),
                        in_=h_sl[:, :gsz * HID])

        for l in range(L):
            linear_phase(l)
            nc.gpsimd.collective_compute(
                "AllGather", mybir.AluOpType.bypass, replica_groups=rg,
                ins=[t_kvin[:, :]], outs=[t_kv[:, :]])
            edge_phase(l)
            finish_layer(l)

        nc.gpsimd.collective_compute(
            "AllGather", mybir.AluOpType.bypass, replica_groups=rg,
            ins=[t_hfin[:, :]], outs=[t_hf[:, :]])

        for b in range(NPB):
            pidx = ep.tile([P, 2 * KBP], i32, tag="pidx")
            nc.sync.dma_start(out=pidx[:], in_=t_pidx[b])
            hs = ep.tile([P, KBP * HID], bf16, tag="hs")
            hs3 = hs[:].rearrange("p (k f) -> p k f", f=HID)
            hd = ep.tile([P, KBP * HID], bf16, tag="hd")
            hd3 = hd[:].rearrange("p (k f) -> p k f", f=HID)
            for j in range(KBP):
                nc.gpsimd.indirect_dma_start(
                    out=hs3[:, j, :], out_offset=None, in_=t_hf[:, :],
                    in_offset=bass.IndirectOffsetOnAxis(
                        ap=pidx[:, j:j + 1], axis=0))
                nc.gpsimd.indirect_dma_start(
                    out=hd3[:, j, :], out_offset=None, in_=t_hf[:, :],
                    in_offset=bass.IndirectOffsetOnAxis(
                        ap=pidx[:, KBP + j:KBP + j + 1], axis=0))
            pm = ep.tile([P, KBP * HID], bf16, tag="pm")
            nc.vector.tensor_tensor(out=pm[:], in0=hs[:], in1=hd[:],
                                    op=ALU.mult)
            dots = ep.tile([P, KBP], f32, tag="dots")
            nc.vector.reduce_sum(
                dots[:], pm[:].rearrange("p (k f) -> p k f", f=HID), axis=X)
            osb = ep.tile([P, KBP], f32, tag="osb")
            nc.scalar.activation(out=osb[:], in_=dots[:], func=ACT.Sigmoid)
            nc.sync.dma_start(
                out=t_out[b * KBP:(b + 1) * KBP, :].rearrange("k p -> p k"),
                in_=osb[:])

    nc.finalize()
    return nc


def kernel(**inputs):
    from concourse.bass_utils import run_bass_kernel_spmd

    pr = _prep(inputs)
    nc = _build(pr)

    in_maps = []
    for c in range(NCORES):
        m = {
            "x_t": pr["xT"][c],
            "rw_t": pr["rwT"][c],
            "w1": pr["W1"],
            "w2": pr["W2"],
            "kvidx": pr["kvidx"][c],
            "onehot": pr["oh"][c],
            "onehot_t": pr["oht"][c],
            "pidx": pr["pidx"][c],
        }
        for l in range(L):
            m[f"wc{l}"] = pr["Wcat"][l]
        if bool(np.any(pr["bcat"] != 0)):
            m["bc_r"] = pr["bcat"][None, :].astype(np.float32)
            m["bc_c"] = pr["bcat"][:, None].astype(np.float32)
        for l in range(L):
            if bool(np.any(pr["bqkvs"][l] != 0)):
                m[f"bqk{l}"] = pr["bqkvs"][l][None, :].astype(np.float32)
            if not (bool(np.all(pr["ln_g"][l] == 1))
                    and bool(np.all(pr["ln_b"][l] == 0))):
                m[f"lng{l}"] = pr["ln_g"][l][None, :].astype(np.float32)
                m[f"lnb{l}"] = pr["ln_b"][l][None, :].astype(np.float32)
        in_maps.append(m)

    kw = {}
    if os.environ.get("KERNEL_TMPDIR"):
        kw["tmpdir"] = os.environ["KERNEL_TMPDIR"]
    res = run_bass_kernel_spmd(
        nc, in_maps, core_ids=list(range(NCORES)),
        trace=bool(int(os.environ.get("KERNEL_TRACE", "0"))), **kw)
    if res.exec_time_ns is not None:
        print(f"HW exec time: {res.exec_time_ns} ns")

    PLOC = pr["PLOC"]
    outs = []
    for c in range(NCORES):
        vals = res.results[c]["out"].reshape(-1)
        outs.append(vals[:pr["plocs"][c]])
    return np.concatenate(outs).astype(np.float32)


# revision 10
# speedup vs baseline: 1.8918x; 1.0309x over previous
import math
import os
from contextlib import ExitStack

import numpy as np

P = 128
HID = 128
HEADS = 4
DH = 32
L = 2
EPS = 1e-5
NCORES = 8
KB = 8
QS = 4
GT = 8
KBP = 8


def _groups(nt, g):
    out = []
    t0 = 0
    while t0 < nt:
        out.append((t0, min(g, nt - t0)))
        t0 += min(g, nt - t0)
    return out


def _prep(inputs):
    import ml_dtypes

    bf = ml_dtypes.bfloat16
    f8 = ml_dtypes.float8_e4m3

    x = np.ascontiguousarray(np.asarray(inputs["x"], dtype=np.float32))
    rw = np.ascontiguousarray(np.asarray(inputs["rw_diag"], dtype=np.float32))
    ei = np.asarray(inputs["edge_index"]).astype(np.int64)
    psrc = np.asarray(inputs["src"]).astype(np.int64)
    pdst = np.asarray(inputs["dst"]).astype(np.int64)

    N = x.shape[0]
    IN_C = x.shape[1]
    RWD = rw.shape[1]
    NT = math.ceil(N / (NCORES * P))
    NLOC = NT * P
    NPADT = NLOC * NCORES

    W_rwse = np.asarray(inputs["W_rwse"], np.float32)
    b_rwse = np.asarray(inputs["b_rwse"], np.float32)
    W_in = np.asarray(inputs["W_in"], np.float32)
    b_in = np.asarray(inputs["b_in"], np.float32)
    W1 = np.ascontiguousarray(W_in[:IN_C]).astype(bf)
    W2 = np.ascontiguousarray(W_rwse @ W_in[IN_C:]).astype(bf)
    bcat = (b_in + b_rwse @ W_in[IN_C:]).astype(np.float32)

    Wq = np.asarray(inputs["Wq"], np.float32)
    Wk = np.asarray(inputs["Wk"], np.float32)
    Wv = np.asarray(inputs["Wv"], np.float32)
    Ws = np.asarray(inputs["Ws"], np.float32)
    bq = np.asarray(inputs["bq"], np.float32)
    bk = np.asarray(inputs["bk"], np.float32)
    bv = np.asarray(inputs["bv"], np.float32)
    bs = np.asarray(inputs["bs"], np.float32)
    ln_g = np.asarray(inputs["ln_g"], np.float32)
    ln_b = np.asarray(inputs["ln_b"], np.float32)

    Wcat = [np.ascontiguousarray(np.concatenate(
        [Wq[l], Wk[l], Wv[l], Ws[l]], axis=1)).astype(bf) for l in range(L)]
    bqkvs = [np.concatenate([bq[l], bk[l], bv[l], bs[l]]) for l in range(L)]

    row = ei[0]
    col = ei[1]
    core_of = col // NLOC
    tile_of = (col % NLOC) // P

    order = np.lexsort((row, tile_of, core_of))
    srow = row[order]
    scol = col[order]

    flat = core_of[order] * NT + tile_of[order]
    cnt = np.bincount(flat, minlength=NCORES * NT).reshape(NCORES, NT)
    tcnt = np.maximum(1, np.ceil(cnt / P).astype(np.int64)).max(axis=0)
    total_t = int(tcnt.sum())
    tcnt[-1] += (-total_t) % KB
    ET = int(tcnt.sum())
    NB = ET // KB

    sub_tile = np.repeat(np.arange(NT), tcnt)
    starts = np.concatenate([[0], np.cumsum(tcnt)])[:-1] * P
    sub_start = np.zeros(ET, bool)
    sub_stop = np.zeros(ET, bool)
    sub_start[starts // P] = True
    sub_stop[(starts // P + tcnt - 1)] = True

    kvidx = np.zeros((NCORES, ET * P), np.int32)
    ohm = np.zeros((NCORES, ET * P, P), bf)
    ohtm = np.zeros((NCORES, ET, P, P), bf)
    gstart = np.concatenate([[0], np.cumsum(cnt.reshape(-1))])
    for c in range(NCORES):
        for t in range(NT):
            g0 = gstart[c * NT + t]
            n = cnt[c, t]
            if n == 0:
                continue
            o = starts[t]
            sl = np.arange(o, o + n)
            kvidx[c, sl] = srow[g0:g0 + n]
            cl = (scol[g0:g0 + n] - c * NLOC).astype(np.int32)
            ohm[c, sl, cl % P] = 1
            ohtm[c, sl // P, cl % P, sl % P] = 1

    kvidx_hw = kvidx.reshape(NCORES, NB, KB, P).transpose(0, 1, 3, 2)
    kvidx_hw = np.ascontiguousarray(kvidx_hw)
    oh_hw = np.ascontiguousarray(
        ohm.reshape(NCORES, NB, KB, P, P).transpose(0, 1, 3, 2, 4)).reshape(
        NCORES, NB, P, KB * P)
    oht_hw = np.ascontiguousarray(
        ohtm.reshape(NCORES, NB, KB, P, P).transpose(0, 1, 3, 2, 4)).reshape(
        NCORES, NB, P, KB * P)

    xs = np.zeros((NCORES, NLOC, IN_C), np.float32)
    rws = np.zeros((NCORES, NLOC, RWD), np.float32)
    for c in range(NCORES):
        lo = c * NLOC
        hi = min(N, lo + NLOC)
        if hi > lo:
            xs[c, :hi - lo] = x[lo:hi]
            rws[c, :hi - lo] = rw[lo:hi]
    xT_hw = np.ascontiguousarray(
        xs.reshape(NCORES, NT, P, IN_C).transpose(0, 1, 3, 2)).astype(bf)
    rwT_hw = np.ascontiguousarray(
        rws.reshape(NCORES, NT, P, RWD).transpose(0, 1, 3, 2)).astype(bf)

    NPAIR = psrc.shape[0]
    PLOC = math.ceil(NPAIR / NCORES)
    plocs = [max(0, min(PLOC, NPAIR - c * PLOC)) for c in range(NCORES)]
    NPS = math.ceil(PLOC / P)
    NPB = math.ceil(NPS / KBP)
    PPAD = NPB * KBP * P
    ps = np.zeros((NCORES, PPAD), np.int32)
    pd = np.zeros((NCORES, PPAD), np.int32)
    for c in range(NCORES):
        n = plocs[c]
        ps[c, :n] = psrc[c * PLOC:c * PLOC + n]
        pd[c, :n] = pdst[c * PLOC:c * PLOC + n]
    pidx_hw = np.zeros((NCORES, NPB, P, 2 * KBP), np.int32)
    pidx_hw[:, :, :, 0:KBP] = ps.reshape(
        NCORES, NPB, KBP, P).transpose(0, 1, 3, 2)
    pidx_hw[:, :, :, KBP:] = pd.reshape(
        NCORES, NPB, KBP, P).transpose(0, 1, 3, 2)

    return dict(
        N=N, IN_C=IN_C, RWD=RWD, NT=NT, NLOC=NLOC, NPADT=NPADT,
        ET=ET, NB=NB, NPB=NPB, PLOC=PLOC, plocs=plocs, NPAIR=NPAIR,
        W1=W1, W2=W2, bcat=bcat, Wcat=Wcat, bqkvs=bqkvs,
        ln_g=ln_g, ln_b=ln_b,
        xT=xT_hw, rwT=rwT_hw, kvidx=kvidx_hw, oh=oh_hw, oht=oht_hw,
        pidx=pidx_hw,
        sub_tile=sub_tile, sub_start=sub_start, sub_stop=sub_stop,
    )


def _build(pr):
    import concourse.bass as bass
    import concourse.bacc as bacc
    import concourse.mybir as mybir
    import concourse.tile as tile

    f32 = mybir.dt.float32
    bf16 = mybir.dt.bfloat16
    fp8 = mybir.dt.float8e4
    i32 = mybir.dt.int32
    ALU = mybir.AluOpType
    ACT = mybir.ActivationFunctionType
    X = mybir.AxisListType.X

    NT, NLOC, NPADT = pr["NT"], pr["NLOC"], pr["NPADT"]
    NB, NPB = pr["NB"], pr["NPB"]
    IN_C, RWD = pr["IN_C"], pr["RWD"]
    scale = 1.0 / math.sqrt(DH)

    bias_nz = bool(np.any(pr["bcat"] != 0))
    qkvs_nz = [bool(np.any(b != 0)) for b in pr["bqkvs"]]
    g_one = [bool(np.all(pr["ln_g"][l] == 1)) for l in range(L)]
    b_zero = [bool(np.all(pr["ln_b"][l] == 0)) for l in range(L)]

    nc = bacc.Bacc(None, num_devices=NCORES)

    t_xT = nc.dram_tensor("x_t", [NT, IN_C, P], bf16, kind="ExternalInput")
    t_rwT = nc.dram_tensor("rw_t", [NT, RWD, P], bf16, kind="ExternalInput")
    t_w1 = nc.dram_tensor("w1", [IN_C, HID], bf16, kind="ExternalInput")
    t_w2 = nc.dram_tensor("w2", [RWD, HID], bf16, kind="ExternalInput")
    t_wc = [nc.dram_tensor(f"wc{l}", [HID, 4 * HID], bf16,
                           kind="ExternalInput") for l in range(L)]
    t_kvi = nc.dram_tensor("kvidx", [NB, P, KB], i32, kind="ExternalInput")
    t_oh = nc.dram_tensor("onehot", [NB, P, KB * P], bf16,
                          kind="ExternalInput")
    t_oht = nc.dram_tensor("onehot_t", [NB, P, KB * P], bf16,
                           kind="ExternalInput")
    t_pidx = nc.dram_tensor("pidx", [NPB, P, 2 * KBP], i32,
                            kind="ExternalInput")

    t_h = nc.dram_tensor("h_loc", [NT, P, HID], bf16, kind="Internal")
    t_kvin = nc.dram_tensor("kv_in", [NLOC, 2 * HID], fp8, kind="Internal")
    t_kv = nc.dram_tensor("kv_full", [NPADT, 2 * HID], fp8,
                          kind="Internal", addr_space="Shared")
    t_hfin = nc.dram_tensor("hf_in", [NLOC, HID], fp8, kind="Internal")
    t_hf = nc.dram_tensor("hf_full", [NPADT, HID], fp8, kind="Internal",
                          addr_space="Shared")
    t_out = nc.dram_tensor("out", [NPB * KBP, P], f32,
                           kind="ExternalOutput")

    rg = [list(range(NCORES))]
    groups = _groups(NT, GT)

    with ExitStack() as ctx:
        tc = ctx.enter_context(tile.TileContext(nc))
        cpool = ctx.enter_context(tc.tile_pool(name="const", bufs=1))
        lp = ctx.enter_context(tc.tile_pool(name="lp", bufs=2))
        ep = ctx.enter_context(tc.tile_pool(name="ep", bufs=4))
        epi = ctx.enter_context(tc.tile_pool(name="epi", bufs=2))
        pp_mm = ctx.enter_context(tc.tile_pool(name="ppm", bufs=3,
                                               space="PSUM"))
        pp_agg = ctx.enter_context(tc.tile_pool(name="ppa", bufs=3,
                                                space="PSUM"))
        pp_qs = ctx.enter_context(tc.tile_pool(name="ppq", bufs=2,
                                               space="PSUM"))

        w1 = cpool.tile([IN_C, HID], bf16)
        nc.sync.dma_start(out=w1[:], in_=t_w1[:, :])
        w2 = cpool.tile([RWD, HID], bf16)
        nc.sync.dma_start(out=w2[:], in_=t_w2[:, :])
        wc = []
        for l in range(L):
            w = cpool.tile([HID, 4 * HID], bf16, name=f"wc{l}")
            nc.sync.dma_start(out=w[:], in_=t_wc[l][:, :])
            wc.append(w)

        sh_all = cpool.tile([P, NT * HID], bf16, name="sh_all")
        yc_all = cpool.tile([P, NT * HID], bf16, name="yc_all")
        q_all = cpool.tile([P, NT * HID], bf16, name="q_all")
        var_all = cpool.tile([P, NT], f32, name="var_all")
        isd_all = cpool.tile([P, NT], f32, name="isd_all")

        def linear_phase(l):
            for (g0, gsz) in groups:
                hT_sl = lp.tile([P, GT * P], bf16, tag="hT")
                h_sl = lp.tile([P, GT * P], bf16, tag="h")
                if l == 0:
                    xT_sl = lp.tile([P, GT * P], bf16, tag="xT")
                    nc.sync.dma_start(
                        out=xT_sl[:, :gsz * P],
                        in_=t_xT[g0:g0 + gsz].rearrange("g c n -> c g n"))
                    rwT_sl = lp.tile([RWD, GT * P], bf16, tag="rwT")
                    nc.sync.dma_start(
                        out=rwT_sl[:, :gsz * P],
                        in_=t_rwT[g0:g0 + gsz].rearrange("g c n -> c g n"))
                    for j in range(gsz):
                        sl = slice(j * P, (j + 1) * P)
                        hT_ps = pp_mm.tile([P, P], f32, tag="mm0")
                        nc.tensor.matmul(out=hT_ps[:], lhsT=w1[:],
                                         rhs=xT_sl[:, sl], start=True,
                                         stop=False)
                        nc.tensor.matmul(out=hT_ps[:], lhsT=w2[:],
                                         rhs=rwT_sl[:, sl], start=False,
                                         stop=True)
                        h_ps = pp_mm.tile([P, P], f32, tag="mm0")
                        nc.tensor.matmul(out=h_ps[:], lhsT=xT_sl[:, sl],
                                         rhs=w1[:], start=True, stop=False)
                        nc.tensor.matmul(out=h_ps[:], lhsT=rwT_sl[:, sl],
                                         rhs=w2[:], start=False, stop=True)
                        nc.scalar.copy(out=hT_sl[:, sl], in_=hT_ps[:])
                        nc.scalar.copy(out=h_sl[:, sl], in_=h_ps[:])
                    if bias_nz:
                        bc_r = cpool.tile([1, HID], f32, name="bc_r")
                        bc_c = cpool.tile([P, 1], f32, name="bc_c")
                        nc.vector.tensor_tensor(
                            out=h_sl[:, :gsz * P].rearrange(
                                "p (g f) -> p g f", f=HID),
                            in0=h_sl[:, :gsz * P].rearrange(
                                "p (g f) -> p g f", f=HID),
                            in1=bc_r[:].to_broadcast([P, gsz, HID]),
                            op=ALU.add)
                        nc.vector.tensor_scalar_add(
                            hT_sl[:, :gsz * P], hT_sl[:, :gsz * P], bc_c[:])
                else:
                    nc.sync.dma_start(
                        out=hT_sl[:, :gsz * P],
                        in_=t_h[g0:g0 + gsz].rearrange("g p f -> (g p) f"),
                        transpose=True)
                    nc.scalar.dma_start(
                        out=h_sl[:, :gsz * P],
                        in_=t_h[g0:g0 + gsz].rearrange("g p f -> p g f"))
                kvs_sl = lp.tile([P, GT * 2 * HID], fp8, tag="kvs")
                for j in range(gsz):
                    t = g0 + j
                    sl = slice(j * P, (j + 1) * P)
                    qk_ps = pp_mm.tile([P, 4 * HID], f32, tag="mm0")
                    nc.tensor.matmul(out=qk_ps[:], lhsT=hT_sl[:, sl],
                                     rhs=wc[l][:], start=True, stop=True)
                    if qkvs_nz[l]:
                        bqk = cpool.tile([1, 4 * HID], f32, name=f"bqk{l}")
                        nc.vector.tensor_tensor(
                            out=qk_ps[:], in0=qk_ps[:],
                            in1=bqk[:].to_broadcast([P, 4 * HID]), op=ALU.add)
                    nc.scalar.copy(out=q_all[:, t * HID:(t + 1) * HID],
                                   in_=qk_ps[:, 0:HID])
                    nc.scalar.copy(
                        out=kvs_sl[:, j * 2 * HID:(j + 1) * 2 * HID],
                        in_=qk_ps[:, HID:3 * HID])
                    nc.vector.tensor_tensor(
                        out=sh_all[:, t * HID:(t + 1) * HID],
                        in0=qk_ps[:, 3 * HID:4 * HID], in1=h_sl[:, sl],
                        op=ALU.add)
                nc.sync.dma_start(
                    out=t_kvin[g0 * P:(g0 + gsz) * P, :].rearrange(
                        "(g p) f -> p g f", p=P),
                    in_=kvs_sl[:, :gsz * 2 * HID])

        def pass1(l, t, agg):
            zsb = epi.tile([P, HEADS], f32, tag="zsb")
            nc.vector.tensor_scalar_max(zsb[:], agg[:, HID:HID + HEADS],
                                        1e-30)
            inv = epi.tile([P, HEADS], f32, tag="inv")
            nc.vector.reciprocal(inv[:], zsb[:])
            y = epi.tile([P, HID], bf16, tag="y")
            nc.vector.tensor_tensor(
                out=y[:].rearrange("p (h d) -> p h d", d=DH),
                in0=agg[:, 0:HID].rearrange("p (h d) -> p h d", d=DH),
                in1=inv[:].unsqueeze(-1).to_broadcast([P, HEADS, DH]),
                op=ALU.mult)
            nc.vector.tensor_tensor(
                out=y[:], in0=y[:], in1=sh_all[:, t * HID:(t + 1) * HID],
                op=ALU.add)
            musum = epi.tile([P, 1], f32, tag="musum")
            nc.vector.reduce_sum(musum[:], y[:], axis=X)
            mu = epi.tile([P, 1], f32, tag="mu")
            nc.vector.tensor_scalar_mul(mu[:], musum[:], 1.0 / HID)
            yc = yc_all[:, t * HID:(t + 1) * HID]
            nc.vector.tensor_scalar_sub(yc, y[:], mu[:])
            junk = epi.tile([P, HID], bf16, tag="junk")
            ssq = epi.tile([P, 1], f32, tag="ssq")
            nc.vector.scalar_tensor_tensor(
                out=junk[:], in0=yc, scalar=1.0, in1=yc,
                op0=ALU.bypass, op1=ALU.mult, accum_out=ssq[:])
            nc.vector.tensor_scalar(
                out=var_all[:, t:t + 1], in0=ssq[:], scalar1=1.0 / HID,
                scalar2=EPS, op0=ALU.mult, op1=ALU.add)

        def finish_group(l, g0, gsz, gen, lng, lnb):
            sdg = epi.tile([P, GT], f32, tag="sdg")
            nc.scalar.activation(out=sdg[:, :gsz],
                                 in_=var_all[:, g0:g0 + gsz], func=ACT.Sqrt)
            nc.vector.reciprocal(isd_all[:, g0:g0 + gsz], sdg[:, :gsz])
            h_sl = lp.tile([P, GT * HID], bf16, tag="ho")
            for j in range(gsz):
                t = g0 + j
                sl = slice(j * HID, (j + 1) * HID)
                ycb = yc_all[:, t * HID:(t + 1) * HID]
                if gen:
                    tmp = epi.tile([P, HID], bf16, tag="tmp")
                    nc.vector.tensor_scalar_mul(
                        tmp[:], ycb, isd_all[:, t:t + 1])
                    nc.vector.tensor_tensor(
                        out=tmp[:], in0=tmp[:],
                        in1=lng[:].to_broadcast([P, HID]), op=ALU.mult)
                    nc.vector.tensor_tensor(
                        out=tmp[:], in0=tmp[:],
                        in1=lnb[:].to_broadcast([P, HID]), op=ALU.add)
                    nc.vector.tensor_scalar_max(h_sl[:, sl], tmp[:], 0.0)
                else:
                    nc.vector.tensor_scalar(
                        out=h_sl[:, sl], in0=ycb,
                        scalar1=isd_all[:, t:t + 1], scalar2=0.0,
                        op0=ALU.mult, op1=ALU.max)
            if l < L - 1:
                nc.sync.dma_start(
                    out=t_h[g0:g0 + gsz].rearrange("g p f -> p g f"),
                    in_=h_sl[:, :gsz * HID])
            else:
                hf_sl = lp.tile([P, GT * HID], fp8, tag="hf8")
                nc.scalar.copy(out=hf_sl[:, :gsz * HID],
                               in_=h_sl[:, :gsz * HID])
                nc.sync.dma_start(
                    out=t_hfin[g0 * P:(g0 + gsz) * P, :].rearrange(
                        "(g p) f -> p g f", p=P),
                    in_=hf_sl[:, :gsz * HID])

        def edge_phase(l):
            sub_tile = pr["sub_tile"]
            sub_start = pr["sub_start"]
            sub_stop = pr["sub_stop"]
            agg_map = {}
            gen = not (g_one[l] and b_zero[l])
            lng = lnb = None
            if gen:
                lng = cpool.tile([1, HID], f32, name=f"lng{l}")
                lnb = cpool.tile([1, HID], f32, name=f"lnb{l}")
            group_of_last = {g0 + gsz - 1: (g0, gsz)
                             for (g0, gsz) in groups}
            for b in range(NB):
                idx = ep.tile([P, KB], i32, tag="idx")
                nc.sync.dma_start(out=idx[:], in_=t_kvi[b])
                oh = ep.tile([P, KB * P], bf16, tag="oh")
                nc.sync.dma_start(out=oh[:], in_=t_oh[b])
                oht = ep.tile([P, KB * P], bf16, tag="oht")
                nc.scalar.dma_start(out=oht[:], in_=t_oht[b])
                oht3 = oht[:].rearrange("p (k n) -> p k n", n=P)
                kvg = ep.tile([P, KB * 2 * HID], bf16, tag="kvg")
                kvg3 = kvg[:].rearrange("p (k f) -> p k f", f=2 * HID)
                qg = ep.tile([P, KB * HID], bf16, tag="qg")
                for j in range(KB):
                    nc.gpsimd.indirect_dma_start(
                        out=kvg3[:, j, :], out_offset=None, in_=t_kv[:, :],
                        in_offset=bass.IndirectOffsetOnAxis(
                            ap=idx[:, j:j + 1], axis=0))
                for j0 in range(0, KB, QS):
                    qs_ps = pp_qs.tile([P, QS * HID], f32, tag="qs")
                    for j in range(j0, j0 + QS):
                        st = b * KB + j
                        t = int(sub_tile[st])
                        nc.tensor.matmul(
                            out=qs_ps[:, (j - j0) * HID:(j - j0 + 1) * HID],
                            lhsT=oht3[:, j, :],
                            rhs=q_all[:, t * HID:(t + 1) * HID],
                            start=True, stop=True)
                    nc.scalar.copy(
                        out=qg[:, j0 * HID:(j0 + QS) * HID], in_=qs_ps[:])
                prod = ep.tile([P, KB * HID], bf16, tag="prod")
                nc.vector.tensor_tensor(
                    out=prod[:].rearrange("p (k f) -> p k f", f=HID),
                    in0=kvg3[:, :, 0:HID],
                    in1=qg[:].rearrange("p (k f) -> p k f", f=HID),
                    op=ALU.mult)
                sc = ep.tile([P, KB * HEADS], f32, tag="sc")
                nc.vector.reduce_sum(
                    sc[:], prod[:].rearrange("p (g d) -> p g d", d=DH),
                    axis=X)
                ext = ep.tile([P, KB * HID], bf16, tag="ext")
                nc.scalar.activation(
                    out=ext[:].rearrange("p (k h d) -> p k h d", h=HEADS,
                                         d=DH),
                    in_=sc[:].rearrange("p (k h) -> p k h", h=HEADS)
                        .unsqueeze(-1).to_broadcast([P, KB, HEADS, DH]),
                    func=ACT.Exp, scale=scale)
                rhs = ep.tile([P, KB * (HID + HEADS)], bf16, tag="rhs")
                rhs3 = rhs[:].rearrange("p (k f) -> p k f", f=HID + HEADS)
                nc.scalar.activation(
                    out=rhs3[:, :, HID:HID + HEADS],
                    in_=sc[:].rearrange("p (k h) -> p k h", h=HEADS),
                    func=ACT.Exp, scale=scale)
                nc.vector.tensor_tensor(
                    out=rhs3[:, :, 0:HID], in0=kvg3[:, :, HID:2 * HID],
                    in1=ext[:].rearrange("p (k f) -> p k f", f=HID),
                    op=ALU.mult)
                oh3 = oh[:].rearrange("p (k n) -> p k n", n=P)
                for j in range(KB):
                    st = b * KB + j
                    t = int(sub_tile[st])
                    if sub_start[st]:
                        agg_map[t] = pp_agg.tile([P, HID + HEADS], f32,
                                                 tag="agg", name="aggt")
                    nc.tensor.matmul(
                        out=agg_map[t][:], lhsT=oh3[:, j, :],
                        rhs=rhs3[:, j, :], start=bool(sub_start[st]),
                        stop=bool(sub_stop[st]))
                    if sub_stop[st]:
                        pass1(l, t, agg_map.pop(t)[:])
                        if t in group_of_last:
                            fg0, fgsz = group_of_last[t]
                            finish_group(l, fg0, fgsz, gen, lng, lnb)

        def finish_layer(l):
            sd = epi.tile([P, NT], f32, tag="sd")
            nc.scalar.activation(out=sd[:], in_=var_all[:], func=ACT.Sqrt)
            nc.vector.reciprocal(isd_all[:], sd[:])
            gen = not (g_one[l] and b_zero[l])
            if gen:
                lng = cpool.tile([1, HID], f32, name=f"lng{l}")
                lnb = cpool.tile([1, HID], f32, name=f"lnb{l}")
            for (g0, gsz) in groups:
                h_sl = lp.tile([P, GT * HID], bf16, tag="ho")
                for j in range(gsz):
                    t = g0 + j
                    sl = slice(j * HID, (j + 1) * HID)
                    ycb = yc_all[:, t * HID:(t + 1) * HID]
                    if gen:
                        tmp = epi.tile([P, HID], bf16, tag="tmp")
                        nc.vector.tensor_scalar_mul(
                            tmp[:], ycb, isd_all[:, t:t + 1])
                        nc.vector.tensor_tensor(
                            out=tmp[:], in0=tmp[:],
                            in1=lng[:].to_broadcast([P, HID]), op=ALU.mult)
                        nc.vector.tensor_tensor(
                            out=tmp[:], in0=tmp[:],
                            in1=lnb[:].to_broadcast([P, HID]), op=ALU.add)
                        nc.vector.tensor_scalar_max(h_sl[:, sl], tmp[:], 0.0)
                    else:
                        nc.vector.tensor_scalar(
                            out=h_sl[:, sl], in0=ycb,
                            scalar1=isd_all[:, t:t + 1], scalar2=0.0,
                            op0=ALU.mult, op1=ALU.max)
                if l < L - 1:
                    nc.sync.dma_start(
                        out=t_h[g0:g0 + gsz].rearrange("g p f -> p g f"),
                        in_=h_sl[:, :gsz * HID])
                else:
                    nc.sync.dma_start(
                        out=t_hfin[g0 * P:(g0 + gsz) * P, :].rearrange(
                            "(g p) f -> p g f", p=P),
                        in_=h_sl[:, :gsz * HID])

        for l in range(L):
            linear_phase(l)
            nc.gpsimd.collective_compute(
                "AllGather", mybir.AluOpType.bypass, replica_groups=rg,
                ins=[t_kvin[:, :]], outs=[t_kv[:, :]])
            edge_phase(l)

        nc.gpsimd.collective_compute(
            "AllGather", mybir.AluOpType.bypass, replica_groups=rg,
            ins=[t_hfin[:, :]], outs=[t_hf[:, :]])

        for b in range(NPB):
            pidx = ep.tile([P, 2 * KBP], i32, tag="pidx")
            nc.sync.dma_start(out=pidx[:], in_=t_pidx[b])
            hs = ep.tile([P, KBP * HID], bf16, tag="hs")
            hs3 = hs[:].rearrange("p (k f) -> p k f", f=HID)
            hd = ep.tile([P, KBP * HID], bf16, tag="hd")
            hd3 = hd[:].rearrange("p (k f) -> p k f", f=HID)
            for j in range(KBP):
                nc.gpsimd.indirect_dma_start(
                    out=hs3[:, j, :], out_offset=None, in_=t_hf[:, :],
                    in_offset=bass.IndirectOffsetOnAxis(
                        ap=pidx[:, j:j + 1], axis=0))
                nc.gpsimd.indirect_dma_start(
                    out=hd3[:, j, :], out_offset=None, in_=t_hf[:, :],
                    in_offset=bass.IndirectOffsetOnAxis(
                        ap=pidx[:, KBP + j:KBP + j + 1], axis=0))
            pm = ep.tile([P, KBP * HID], bf16, tag="pm")
            nc.vector.tensor_tensor(out=pm[:], in0=hs[:], in1=hd[:],
                                    op=ALU.mult)
            dots = ep.tile([P, KBP], f32, tag="dots")
            nc.vector.reduce_sum(
                dots[:], pm[:].rearrange("p (k f) -> p k f", f=HID), axis=X)
            osb = ep.tile([P, KBP], f32, tag="osb")
            nc.scalar.activation(out=osb[:], in_=dots[:], func=ACT.Sigmoid)
            nc.sync.dma_start(
                out=t_out[b * KBP:(b + 1) * KBP, :].rearrange("k p -> p k"),
                in_=osb[:])

    nc.finalize()
    return nc


def kernel(**inputs):
    from concourse.bass_utils import run_bass_kernel_spmd

    pr = _prep(inputs)
    nc = _build(pr)

    in_maps = []
    for c in range(NCORES):
        m = {
            "x_t": pr["xT"][c],
            "rw_t": pr["rwT"][c],
            "w1": pr["W1"],
            "w2": pr["W2"],
            "kvidx": pr["kvidx"][c],
            "onehot": pr["oh"][c],
            "onehot_t": pr["oht"][c],
            "pidx": pr["pidx"][c],
        }
        for l in range(L):
            m[f"wc{l}"] = pr["Wcat"][l]
        if bool(np.any(pr["bcat"] != 0)):
            m["bc_r"] = pr["bcat"][None, :].astype(np.float32)
            m["bc_c"] = pr["bcat"][:, None].astype(np.float32)
        for l in range(L):
            if bool(np.any(pr["bqkvs"][l] != 0)):
                m[f"bqk{l}"] = pr["bqkvs"][l][None, :].astype(np.float32)
            if not (bool(np.all(pr["ln_g"][l] == 1))
                    and bool(np.all(pr["ln_b"][l] == 0))):
                m[f"lng{l}"] = pr["ln_g"][l][None, :].astype(np.float32)
                m[f"lnb{l}"] = pr["ln_b"][l][None, :].astype(np.float32)
        in_maps.append(m)

    kw = {}
    if os.environ.get("KERNEL_TMPDIR"):
        kw["tmpdir"] = os.environ["KERNEL_TMPDIR"]
    res = run_bass_kernel_spmd(
        nc, in_maps, core_ids=list(range(NCORES)),
        trace=bool(int(os.environ.get("KERNEL_TRACE", "0"))), **kw)
    if res.exec_time_ns is not None:
        print(f"HW exec time: {res.exec_time_ns} ns")

    PLOC = pr["PLOC"]
    outs = []
    for c in range(NCORES):
        vals = res.results[c]["out"].reshape(-1)
        outs.append(vals[:pr["plocs"][c]])
    return np.concatenate(outs).astype(np.float32)


# revision 17
# speedup vs baseline: 1.9632x; 1.0377x over previous
import math
import os
from contextlib import ExitStack

import numpy as np

P = 128
HID = 128
HEADS = 4
DH = 32
L = 2
EPS = 1e-5
NCORES = 8
KB = 8
QS = 4
GT = 8
KBP = 8


def _groups(nt, g):
    out = []
    t0 = 0
    while t0 < nt:
        out.append((t0, min(g, nt - t0)))
        t0 += min(g, nt - t0)
    return out


def _prep(inputs):
    import ml_dtypes

    bf = ml_dtypes.bfloat16
    f8 = ml_dtypes.float8_e4m3

    x = np.ascontiguousarray(np.asarray(inputs["x"], dtype=np.float32))
    rw = np.ascontiguousarray(np.asarray(inputs["rw_diag"], dtype=np.float32))
    ei = np.asarray(inputs["edge_index"]).astype(np.int64)
    psrc = np.asarray(inputs["src"]).astype(np.int64)
    pdst = np.asarray(inputs["dst"]).astype(np.int64)

    N = x.shape[0]
    IN_C = x.shape[1]
    RWD = rw.shape[1]
    NT = math.ceil(N / (NCORES * P))
    NLOC = NT * P
    NPADT = NLOC * NCORES

    W_rwse = np.asarray(inputs["W_rwse"], np.float32)
    b_rwse = np.asarray(inputs["b_rwse"], np.float32)
    W_in = np.asarray(inputs["W_in"], np.float32)
    b_in = np.asarray(inputs["b_in"], np.float32)
    W1 = np.ascontiguousarray(W_in[:IN_C]).astype(bf)
    W2 = np.ascontiguousarray(W_rwse @ W_in[IN_C:]).astype(bf)
    bcat = (b_in + b_rwse @ W_in[IN_C:]).astype(np.float32)

    Wq = np.asarray(inputs["Wq"], np.float32)
    Wk = np.asarray(inputs["Wk"], np.float32)
    Wv = np.asarray(inputs["Wv"], np.float32)
    Ws = np.asarray(inputs["Ws"], np.float32)
    bq = np.asarray(inputs["bq"], np.float32)
    bk = np.asarray(inputs["bk"], np.float32)
    bv = np.asarray(inputs["bv"], np.float32)
    bs = np.asarray(inputs["bs"], np.float32)
    ln_g = np.asarray(inputs["ln_g"], np.float32)
    ln_b = np.asarray(inputs["ln_b"], np.float32)

    Wcat = [np.ascontiguousarray(np.concatenate(
        [Wq[l], Wk[l], Wv[l], Ws[l]], axis=1)).astype(bf) for l in range(L)]
    bqkvs = [np.concatenate([bq[l], bk[l], bv[l], bs[l]]) for l in range(L)]

    row = ei[0]
    col = ei[1]
    core_of = col // NLOC
    tile_of = (col % NLOC) // P

    order = np.lexsort((row, tile_of, core_of))
    srow = row[order]
    scol = col[order]

    flat = core_of[order] * NT + tile_of[order]
    cnt = np.bincount(flat, minlength=NCORES * NT).reshape(NCORES, NT)
    perm = np.argsort(-cnt, axis=1, kind="stable")
    inv_perm = np.argsort(perm, axis=1)
    cnt_s = np.take_along_axis(cnt, perm, axis=1)
    tcnt = np.maximum(1, np.ceil(cnt_s / P).astype(np.int64)).max(axis=0)
    total_t = int(tcnt.sum())
    tcnt[-1] += (-total_t) % KB
    ET = int(tcnt.sum())
    NB = ET // KB

    def slot_row(g):
        gc = g // NLOC
        gl = g % NLOC
        return gc * NLOC + inv_perm[gc, gl // P] * P + gl % P

    sub_tile = np.repeat(np.arange(NT), tcnt)
    starts = np.concatenate([[0], np.cumsum(tcnt)])[:-1] * P
    sub_start = np.zeros(ET, bool)
    sub_stop = np.zeros(ET, bool)
    sub_start[starts // P] = True
    sub_stop[(starts // P + tcnt - 1)] = True

    msrow = slot_row(srow)
    kvidx = np.zeros((NCORES, ET * P), np.int32)
    ohm = np.zeros((NCORES, ET * P, P), bf)
    ohtm = np.zeros((NCORES, ET, P, P), bf)
    gstart = np.concatenate([[0], np.cumsum(cnt.reshape(-1))])
    for c in range(NCORES):
        for s in range(NT):
            t = perm[c, s]
            g0 = gstart[c * NT + t]
            n = cnt[c, t]
            if n == 0:
                continue
            o = starts[s]
            sl = np.arange(o, o + n)
            kvidx[c, sl] = msrow[g0:g0 + n]
            cl = (scol[g0:g0 + n] - c * NLOC).astype(np.int32)
            ohm[c, sl, cl % P] = 1
            ohtm[c, sl // P, cl % P, sl % P] = 1

    kvidx_hw = kvidx.reshape(NCORES, NB, KB, P).transpose(0, 1, 3, 2)
    kvidx_hw = np.ascontiguousarray(kvidx_hw)
    oh_hw = np.ascontiguousarray(
        ohm.reshape(NCORES, NB, KB, P, P).transpose(0, 1, 3, 2, 4)).reshape(
        NCORES, NB, P, KB * P)
    oht_hw = np.ascontiguousarray(
        ohtm.reshape(NCORES, NB, KB, P, P).transpose(0, 1, 3, 2, 4)).reshape(
        NCORES, NB, P, KB * P)

    xs = np.zeros((NCORES, NLOC, IN_C), np.float32)
    rws = np.zeros((NCORES, NLOC, RWD), np.float32)
    for c in range(NCORES):
        lo = c * NLOC
        hi = min(N, lo + NLOC)
        if hi > lo:
            xs[c, :hi - lo] = x[lo:hi]
            rws[c, :hi - lo] = rw[lo:hi]
    ci = np.arange(NCORES)[:, None]
    xT_hw = np.ascontiguousarray(
        xs.reshape(NCORES, NT, P, IN_C)[ci, perm].transpose(
            0, 1, 3, 2)).astype(bf)
    rwT_hw = np.ascontiguousarray(
        rws.reshape(NCORES, NT, P, RWD)[ci, perm].transpose(
            0, 1, 3, 2)).astype(bf)

    NPAIR = psrc.shape[0]
    PLOC = math.ceil(NPAIR / NCORES)
    plocs = [max(0, min(PLOC, NPAIR - c * PLOC)) for c in range(NCORES)]
    NPS = math.ceil(PLOC / P)
    NPB = math.ceil(NPS / KBP)
    PPAD = NPB * KBP * P
    ps = np.zeros((NCORES, PPAD), np.int32)
    pd = np.zeros((NCORES, PPAD), np.int32)
    mpsrc = slot_row(psrc)
    mpdst = slot_row(pdst)
    for c in range(NCORES):
        n = plocs[c]
        ps[c, :n] = mpsrc[c * PLOC:c * PLOC + n]
        pd[c, :n] = mpdst[c * PLOC:c * PLOC + n]
    pidx_hw = np.zeros((NCORES, NPB, P, 2 * KBP), np.int32)
    pidx_hw[:, :, :, 0:KBP] = ps.reshape(
        NCORES, NPB, KBP, P).transpose(0, 1, 3, 2)
    pidx_hw[:, :, :, KBP:] = pd.reshape(
        NCORES, NPB, KBP, P).transpose(0, 1, 3, 2)

    return dict(
        N=N, IN_C=IN_C, RWD=RWD, NT=NT, NLOC=NLOC, NPADT=NPADT,
        ET=ET, NB=NB, NPB=NPB, PLOC=PLOC, plocs=plocs, NPAIR=NPAIR,
        W1=W1, W2=W2, bcat=bcat, Wcat=Wcat, bqkvs=bqkvs,
        ln_g=ln_g, ln_b=ln_b,
        xT=xT_hw, rwT=rwT_hw, kvidx=kvidx_hw, oh=oh_hw, oht=oht_hw,
        pidx=pidx_hw,
        sub_tile=sub_tile, sub_start=sub_start, sub_stop=sub_stop,
    )


def _build(pr):
    import concourse.bass as bass
    import concourse.bacc as bacc
    import concourse.mybir as mybir
    import concourse.tile as tile

    f32 = mybir.dt.float32
    bf16 = mybir.dt.bfloat16
    fp8 = mybir.dt.float8e4
    i32 = mybir.dt.int32
    ALU = mybir.AluOpType
    ACT = mybir.ActivationFunctionType
    X = mybir.AxisListType.X

    NT, NLOC, NPADT = pr["NT"], pr["NLOC"], pr["NPADT"]
    NB, NPB = pr["NB"], pr["NPB"]
    IN_C, RWD = pr["IN_C"], pr["RWD"]
    scale = 1.0 / math.sqrt(DH)

    bias_nz = bool(np.any(pr["bcat"] != 0))
    qkvs_nz = [bool(np.any(b != 0)) for b in pr["bqkvs"]]
    g_one = [bool(np.all(pr["ln_g"][l] == 1)) for l in range(L)]
    b_zero = [bool(np.all(pr["ln_b"][l] == 0)) for l in range(L)]

    nc = bacc.Bacc(None, num_devices=NCORES)

    t_xT = nc.dram_tensor("x_t", [NT, IN_C, P], bf16, kind="ExternalInput")
    t_rwT = nc.dram_tensor("rw_t", [NT, RWD, P], bf16, kind="ExternalInput")
    t_w1 = nc.dram_tensor("w1", [IN_C, HID], bf16, kind="ExternalInput")
    t_w2 = nc.dram_tensor("w2", [RWD, HID], bf16, kind="ExternalInput")
    t_wc = [nc.dram_tensor(f"wc{l}", [HID, 4 * HID], bf16,
                           kind="ExternalInput") for l in range(L)]
    t_kvi = nc.dram_tensor("kvidx", [NB, P, KB], i32, kind="ExternalInput")
    t_oh = nc.dram_tensor("onehot", [NB, P, KB * P], bf16,
                          kind="ExternalInput")
    t_oht = nc.dram_tensor("onehot_t", [NB, P, KB * P], bf16,
                           kind="ExternalInput")
    t_pidx = nc.dram_tensor("pidx", [NPB, P, 2 * KBP], i32,
                            kind="ExternalInput")

    t_h = nc.dram_tensor("h_loc", [NT, P, HID], bf16, kind="Internal")
    t_kvin = nc.dram_tensor("kv_in", [NLOC, 2 * HID], fp8, kind="Internal")
    t_kv = nc.dram_tensor("kv_full", [NPADT, 2 * HID], fp8,
                          kind="Internal", addr_space="Shared")
    t_hfin = nc.dram_tensor("hf_in", [NLOC, HID], fp8, kind="Internal")
    t_hf = nc.dram_tensor("hf_full", [NPADT, HID], fp8, kind="Internal",
                          addr_space="Shared")
    t_out = nc.dram_tensor("out", [NPB * KBP, P], f32,
                           kind="ExternalOutput")

    rg = [list(range(NCORES))]
    groups = _groups(NT, GT)

    with ExitStack() as ctx:
        tc = ctx.enter_context(tile.TileContext(nc))
        cpool = ctx.enter_context(tc.tile_pool(name="const", bufs=1))
        lp = ctx.enter_context(tc.tile_pool(name="lp", bufs=2))
        ep = ctx.enter_context(tc.tile_pool(name="ep", bufs=4))
        epi = ctx.enter_context(tc.tile_pool(name="epi", bufs=2))
        pp_mm = ctx.enter_context(tc.tile_pool(name="ppm", bufs=3,
                                               space="PSUM"))
        pp_agg = ctx.enter_context(tc.tile_pool(name="ppa", bufs=3,
                                                space="PSUM"))
        pp_qs = ctx.enter_context(tc.tile_pool(name="ppq", bufs=2,
                                               space="PSUM"))

        w1 = cpool.tile([IN_C, HID], bf16)
        nc.sync.dma_start(out=w1[:], in_=t_w1[:, :])
        w2 = cpool.tile([RWD, HID], bf16)
        nc.sync.dma_start(out=w2[:], in_=t_w2[:, :])
        wc = []
        for l in range(L):
            w = cpool.tile([HID, 4 * HID], bf16, name=f"wc{l}")
            nc.sync.dma_start(out=w[:], in_=t_wc[l][:, :])
            wc.append(w)

        sh_all = cpool.tile([P, NT * HID], bf16, name="sh_all")
        yc_all = cpool.tile([P, NT * HID], bf16, name="yc_all")
        q_all = cpool.tile([P, NT * HID], bf16, name="q_all")
        var_all = cpool.tile([P, NT], f32, name="var_all")
        isd_all = cpool.tile([P, NT], f32, name="isd_all")

        def linear_phase(l):
            for (g0, gsz) in groups:
                hT_sl = lp.tile([P, GT * P], bf16, tag="hT")
                h_sl = lp.tile([P, GT * P], bf16, tag="h")
                if l == 0:
                    xT_sl = lp.tile([P, GT * P], bf16, tag="xT")
                    nc.sync.dma_start(
                        out=xT_sl[:, :gsz * P],
                        in_=t_xT[g0:g0 + gsz].rearrange("g c n -> c g n"))
                    rwT_sl = lp.tile([RWD, GT * P], bf16, tag="rwT")
                    nc.sync.dma_start(
                        out=rwT_sl[:, :gsz * P],
                        in_=t_rwT[g0:g0 + gsz].rearrange("g c n -> c g n"))
                    for j in range(gsz):
                        sl = slice(j * P, (j + 1) * P)
                        hT_ps = pp_mm.tile([P, P], f32, tag="mm0")
                        nc.tensor.matmul(out=hT_ps[:], lhsT=w1[:],
                                         rhs=xT_sl[:, sl], start=True,
                                         stop=False)
                        nc.tensor.matmul(out=hT_ps[:], lhsT=w2[:],
                                         rhs=rwT_sl[:, sl], start=False,
                                         stop=True)
                        h_ps = pp_mm.tile([P, P], f32, tag="mm0")
                        nc.tensor.matmul(out=h_ps[:], lhsT=xT_sl[:, sl],
                                         rhs=w1[:], start=True, stop=False)
                        nc.tensor.matmul(out=h_ps[:], lhsT=rwT_sl[:, sl],
                                         rhs=w2[:], start=False, stop=True)
                        nc.scalar.copy(out=hT_sl[:, sl], in_=hT_ps[:])
                        nc.vector.tensor_copy(h_sl[:, sl], h_ps[:])
                    if bias_nz:
                        bc_r = cpool.tile([1, HID], f32, name="bc_r")
                        bc_c = cpool.tile([P, 1], f32, name="bc_c")
                        nc.vector.tensor_tensor(
                            out=h_sl[:, :gsz * P].rearrange(
                                "p (g f) -> p g f", f=HID),
                            in0=h_sl[:, :gsz * P].rearrange(
                                "p (g f) -> p g f", f=HID),
                            in1=bc_r[:].to_broadcast([P, gsz, HID]),
                            op=ALU.add)
                        nc.vector.tensor_scalar_add(
                            hT_sl[:, :gsz * P], hT_sl[:, :gsz * P], bc_c[:])
                else:
                    nc.sync.dma_start(
                        out=hT_sl[:, :gsz * P],
                        in_=t_h[g0:g0 + gsz].rearrange("g p f -> (g p) f"),
                        transpose=True)
                    nc.scalar.dma_start(
                        out=h_sl[:, :gsz * P],
                        in_=t_h[g0:g0 + gsz].rearrange("g p f -> p g f"))
                kvs_sl = lp.tile([P, GT * 2 * HID], fp8, tag="kvs")
                for j in range(gsz):
                    t = g0 + j
                    sl = slice(j * P, (j + 1) * P)
                    qk_ps = pp_mm.tile([P, 4 * HID], f32, tag="mm0")
                    nc.tensor.matmul(out=qk_ps[:], lhsT=hT_sl[:, sl],
                                     rhs=wc[l][:], start=True, stop=True)
                    if qkvs_nz[l]:
                        bqk = cpool.tile([1, 4 * HID], f32, name=f"bqk{l}")
                        nc.vector.tensor_tensor(
                            out=qk_ps[:], in0=qk_ps[:],
                            in1=bqk[:].to_broadcast([P, 4 * HID]), op=ALU.add)
                    nc.scalar.copy(out=q_all[:, t * HID:(t + 1) * HID],
                                   in_=qk_ps[:, 0:HID])
                    nc.scalar.copy(
                        out=kvs_sl[:, j * 2 * HID:(j + 1) * 2 * HID],
                        in_=qk_ps[:, HID:3 * HID])
                    nc.vector.tensor_tensor(
                        out=sh_all[:, t * HID:(t + 1) * HID],
                        in0=qk_ps[:, 3 * HID:4 * HID], in1=h_sl[:, sl],
                        op=ALU.add)
                nc.sync.dma_start(
                    out=t_kvin[g0 * P:(g0 + gsz) * P, :].rearrange(
                        "(g p) f -> p g f", p=P),
                    in_=kvs_sl[:, :gsz * 2 * HID])

        def pass1(l, t, agg):
            zsb = epi.tile([P, HEADS], f32, tag="zsb")
            nc.vector.tensor_scalar_max(zsb[:], agg[:, HID:HID + HEADS],
                                        1e-30)
            inv = epi.tile([P, HEADS], f32, tag="inv")
            nc.vector.reciprocal(inv[:], zsb[:])
            y = epi.tile([P, HID], bf16, tag="y")
            nc.vector.tensor_tensor(
                out=y[:].rearrange("p (h d) -> p h d", d=DH),
                in0=agg[:, 0:HID].rearrange("p (h d) -> p h d", d=DH),
                in1=inv[:].unsqueeze(-1).to_broadcast([P, HEADS, DH]),
                op=ALU.mult)
            nc.vector.tensor_tensor(
                out=y[:], in0=y[:], in1=sh_all[:, t * HID:(t + 1) * HID],
                op=ALU.add)
            musum = epi.tile([P, 1], f32, tag="musum")
            nc.vector.reduce_sum(musum[:], y[:], axis=X)
            mu = epi.tile([P, 1], f32, tag="mu")
            nc.vector.tensor_scalar_mul(mu[:], musum[:], 1.0 / HID)
            yc = yc_all[:, t * HID:(t + 1) * HID]
            nc.vector.tensor_scalar_sub(yc, y[:], mu[:])
            junk = epi.tile([P, HID], bf16, tag="junk")
            ssq = epi.tile([P, 1], f32, tag="ssq")
            nc.vector.scalar_tensor_tensor(
                out=junk[:], in0=yc, scalar=1.0, in1=yc,
                op0=ALU.bypass, op1=ALU.mult, accum_out=ssq[:])
            nc.vector.tensor_scalar(
                out=var_all[:, t:t + 1], in0=ssq[:], scalar1=1.0 / HID,
                scalar2=EPS, op0=ALU.mult, op1=ALU.add)

        def finish_group(l, g0, gsz, gen, lng, lnb):
            sdg = epi.tile([P, GT], f32, tag="sdg")
            nc.scalar.activation(out=sdg[:, :gsz],
                                 in_=var_all[:, g0:g0 + gsz], func=ACT.Sqrt)
            nc.vector.reciprocal(isd_all[:, g0:g0 + gsz], sdg[:, :gsz])
            h_sl = lp.tile([P, GT * HID], bf16, tag="ho")
            for j in range(gsz):
                t = g0 + j
                sl = slice(j * HID, (j + 1) * HID)
                ycb = yc_all[:, t * HID:(t + 1) * HID]
                if gen:
                    tmp = epi.tile([P, HID], bf16, tag="tmp")
                    nc.vector.tensor_scalar_mul(
                        tmp[:], ycb, isd_all[:, t:t + 1])
                    nc.vector.tensor_tensor(
                        out=tmp[:], in0=tmp[:],
                        in1=lng[:].to_broadcast([P, HID]), op=ALU.mult)
                    nc.vector.tensor_tensor(
                        out=tmp[:], in0=tmp[:],
                        in1=lnb[:].to_broadcast([P, HID]), op=ALU.add)
                    nc.vector.tensor_scalar_max(h_sl[:, sl], tmp[:], 0.0)
                else:
                    nc.vector.tensor_scalar(
                        out=h_sl[:, sl], in0=ycb,
                        scalar1=isd_all[:, t:t + 1], scalar2=0.0,
                        op0=ALU.mult, op1=ALU.max)
            if l < L - 1:
                nc.sync.dma_start(
                    out=t_h[g0:g0 + gsz].rearrange("g p f -> p g f"),
                    in_=h_sl[:, :gsz * HID])
            else:
                hf_sl = lp.tile([P, GT * HID], fp8, tag="hf8")
                nc.scalar.copy(out=hf_sl[:, :gsz * HID],
                               in_=h_sl[:, :gsz * HID])
                nc.sync.dma_start(
                    out=t_hfin[g0 * P:(g0 + gsz) * P, :].rearrange(
                        "(g p) f -> p g f", p=P),
                    in_=hf_sl[:, :gsz * HID])

        def edge_phase(l):
            sub_tile = pr["sub_tile"]
            sub_start = pr["sub_start"]
            sub_stop = pr["sub_stop"]
            agg_map = {}
            gen = not (g_one[l] and b_zero[l])
            lng = lnb = None
            if gen:
                lng = cpool.tile([1, HID], f32, name=f"lng{l}")
                lnb = cpool.tile([1, HID], f32, name=f"lnb{l}")
            group_of_last = {g0 + gsz - 1: (g0, gsz)
                             for (g0, gsz) in groups}
            for b in range(NB):
                idx = ep.tile([P, KB], i32, tag="idx")
                nc.sync.dma_start(out=idx[:], in_=t_kvi[b])
                oh = ep.tile([P, KB * P], bf16, tag="oh")
                nc.sync.dma_start(out=oh[:], in_=t_oh[b])
                oht = ep.tile([P, KB * P], bf16, tag="oht")
                nc.scalar.dma_start(out=oht[:], in_=t_oht[b])
                oht3 = oht[:].rearrange("p (k n) -> p k n", n=P)
                kvg = ep.tile([P, KB * 2 * HID], bf16, tag="kvg")
                kvg3 = kvg[:].rearrange("p (k f) -> p k f", f=2 * HID)
                qg = ep.tile([P, KB * HID], bf16, tag="qg")
                for j in range(KB):
                    nc.gpsimd.indirect_dma_start(
                        out=kvg3[:, j, :], out_offset=None, in_=t_kv[:, :],
                        in_offset=bass.IndirectOffsetOnAxis(
                            ap=idx[:, j:j + 1], axis=0))
                for j0 in range(0, KB, QS):
                    qs_ps = pp_qs.tile([P, QS * HID], f32, tag="qs")
                    for j in range(j0, j0 + QS):
                        st = b * KB + j
                        t = int(sub_tile[st])
                        nc.tensor.matmul(
                            out=qs_ps[:, (j - j0) * HID:(j - j0 + 1) * HID],
                            lhsT=oht3[:, j, :],
                            rhs=q_all[:, t * HID:(t + 1) * HID],
                            start=True, stop=True)
                    nc.scalar.copy(
                        out=qg[:, j0 * HID:(j0 + QS) * HID], in_=qs_ps[:])
                prod = ep.tile([P, KB * HID], bf16, tag="prod")
                nc.vector.tensor_tensor(
                    out=prod[:].rearrange("p (k f) -> p k f", f=HID),
                    in0=kvg3[:, :, 0:HID],
                    in1=qg[:].rearrange("p (k f) -> p k f", f=HID),
                    op=ALU.mult)
                sc = ep.tile([P, KB * HEADS], f32, tag="sc")
                nc.vector.reduce_sum(
                    sc[:], prod[:].rearrange("p (g d) -> p g d", d=DH),
                    axis=X)
                ext = ep.tile([P, KB * HID], bf16, tag="ext")
                nc.scalar.activation(
                    out=ext[:].rearrange("p (k h d) -> p k h d", h=HEADS,
                                         d=DH),
                    in_=sc[:].rearrange("p (k h) -> p k h", h=HEADS)
                        .unsqueeze(-1).to_broadcast([P, KB, HEADS, DH]),
                    func=ACT.Exp, scale=scale)
                rhs = ep.tile([P, KB * (HID + HEADS)], bf16, tag="rhs")
                rhs3 = rhs[:].rearrange("p (k f) -> p k f", f=HID + HEADS)
                nc.scalar.activation(
                    out=rhs3[:, :, HID:HID + HEADS],
                    in_=sc[:].rearrange("p (k h) -> p k h", h=HEADS),
                    func=ACT.Exp, scale=scale)
                nc.vector.tensor_tensor(
                    out=rhs3[:, :, 0:HID], in0=kvg3[:, :, HID:2 * HID],
                    in1=ext[:].rearrange("p (k f) -> p k f", f=HID),
                    op=ALU.mult)
                oh3 = oh[:].rearrange("p (k n) -> p k n", n=P)
                for j in range(KB):
                    st = b * KB + j
                    t = int(sub_tile[st])
                    if sub_start[st]:
                        agg_map[t] = pp_agg.tile([P, HID + HEADS], f32,
                                                 tag="agg", name="aggt")
                    nc.tensor.matmul(
                        out=agg_map[t][:], lhsT=oh3[:, j, :],
                        rhs=rhs3[:, j, :], start=bool(sub_start[st]),
                        stop=bool(sub_stop[st]))
                    if sub_stop[st]:
                        pass1(l, t, agg_map.pop(t)[:])
                        if t in group_of_last:
                            fg0, fgsz = group_of_last[t]
                            finish_group(l, fg0, fgsz, gen, lng, lnb)

        def finish_layer(l):
            sd = epi.tile([P, NT], f32, tag="sd")
            nc.scalar.activation(out=sd[:], in_=var_all[:], func=ACT.Sqrt)
            nc.vector.reciprocal(isd_all[:], sd[:])
            gen = not (g_one[l] and b_zero[l])
            if gen:
                lng = cpool.tile([1, HID], f32, name=f"lng{l}")
                lnb = cpool.tile([1, HID], f32, name=f"lnb{l}")
            for (g0, gsz) in groups:
                h_sl = lp.tile([P, GT * HID], bf16, tag="ho")
                for j in range(gsz):
                    t = g0 + j
                    sl = slice(j * HID, (j + 1) * HID)
                    ycb = yc_all[:, t * HID:(t + 1) * HID]
                    if gen:
                        tmp = epi.tile([P, HID], bf16, tag="tmp")
                        nc.vector.tensor_scalar_mul(
                            tmp[:], ycb, isd_all[:, t:t + 1])
                        nc.vector.tensor_tensor(
                            out=tmp[:], in0=tmp[:],
                            in1=lng[:].to_broadcast([P, HID]), op=ALU.mult)
                        nc.vector.tensor_tensor(
                            out=tmp[:], in0=tmp[:],
                            in1=lnb[:].to_broadcast([P, HID]), op=ALU.add)
                        nc.vector.tensor_scalar_max(h_sl[:, sl], tmp[:], 0.0)
                    else:
                        nc.vector.tensor_scalar(
                            out=h_sl[:, sl], in0=ycb,
                            scalar1=isd_all[:, t:t + 1], scalar2=0.0,
                            op0=ALU.mult, op1=ALU.max)
                if l < L - 1:
                    nc.sync.dma_start(
                        out=t_h[g0:g0 + gsz].rearrange("g p f -> p g f"),
                        in_=h_sl[:, :gsz * HID])
                else:
                    nc.sync.dma_start(
                        out=t_hfin[g0 * P:(g0 + gsz) * P, :].rearrange(
                            "(g p) f -> p g f", p=P),
                        in_=h_sl[:, :gsz * HID])

        for l in range(L):
            linear_phase(l)
            nc.gpsimd.collective_compute(
                "AllGather", mybir.AluOpType.bypass, replica_groups=rg,
                ins=[t_kvin[:, :]], outs=[t_kv[:, :]])
            edge_phase(l)

        nc.gpsimd.collective_compute(
            "AllGather", mybir.AluOpType.bypass, replica_groups=rg,
            ins=[t_hfin[:, :]], outs=[t_hf[:, :]])

        for b in range(NPB):
            pidx = ep.tile([P, 2 * KBP], i32, tag="pidx")
            nc.sync.dma_start(out=pidx[:], in_=t_pidx[b])
            hs = ep.tile([P, KBP * HID], bf16, tag="hs")
            hs3 = hs[:].rearrange("p (k f) -> p k f", f=HID)
            hd = ep.tile([P, KBP * HID], bf16, tag="hd")
            hd3 = hd[:].rearrange("p (k f) -> p k f", f=HID)
            for j in range(KBP):
                nc.gpsimd.indirect_dma_start(
                    out=hs3[:, j, :], out_offset=None, in_=t_hf[:, :],
                    in_offset=bass.IndirectOffsetOnAxis(
                        ap=pidx[:, j:j + 1], axis=0))
                nc.gpsimd.indirect_dma_start(
                    out=hd3[:, j, :], out_offset=None, in_=t_hf[:, :],
                    in_offset=bass.IndirectOffsetOnAxis(
                        ap=pidx[:, KBP + j:KBP + j + 1], axis=0))
            pm = ep.tile([P, KBP * HID], bf16, tag="pm")
            nc.vector.tensor_tensor(out=pm[:], in0=hs[:], in1=hd[:],
                                    op=ALU.mult)
            dots = ep.tile([P, KBP], f32, tag="dots")
            nc.vector.reduce_sum(
                dots[:], pm[:].rearrange("p (k f) -> p k f", f=HID), axis=X)
            osb = ep.tile([P, KBP], f32, tag="osb")
            nc.scalar.activation(out=osb[:], in_=dots[:], func=ACT.Sigmoid)
            nc.sync.dma_start(
                out=t_out[b * KBP:(b + 1) * KBP, :].rearrange("k p -> p k"),
                in_=osb[:])

    nc.finalize()
    return nc


def kernel(**inputs):
    from concourse.bass_utils import run_bass_kernel_spmd

    pr = _prep(inputs)
    nc = _build(pr)

    in_maps = []
    for c in range(NCORES):
        m = {
            "x_t": pr["xT"][c],
            "rw_t": pr["rwT"][c],
            "w1": pr["W1"],
            "w2": pr["W2"],
            "kvidx": pr["kvidx"][c],
            "onehot": pr["oh"][c],
            "onehot_t": pr["oht"][c],
            "pidx": pr["pidx"][c],
        }
        for l in range(L):
            m[f"wc{l}"] = pr["Wcat"][l]
        if bool(np.any(pr["bcat"] != 0)):
            m["bc_r"] = pr["bcat"][None, :].astype(np.float32)
            m["bc_c"] = pr["bcat"][:, None].astype(np.float32)
        for l in range(L):
            if bool(np.any(pr["bqkvs"][l] != 0)):
                m[f"bqk{l}"] = pr["bqkvs"][l][None, :].astype(np.float32)
            if not (bool(np.all(pr["ln_g"][l] == 1))
                    and bool(np.all(pr["ln_b"][l] == 0))):
                m[f"lng{l}"] = pr["ln_g"][l][None, :].astype(np.float32)
                m[f"lnb{l}"] = pr["ln_b"][l][None, :].astype(np.float32)
        in_maps.append(m)

    kw = {}
    if os.environ.get("KERNEL_TMPDIR"):
        kw["tmpdir"] = os.environ["KERNEL_TMPDIR"]
    res = run_bass_kernel_spmd(
        nc, in_maps, core_ids=list(range(NCORES)),
        trace=bool(int(os.environ.get("KERNEL_TRACE", "0"))), **kw)
    if res.exec_time_ns is not None:
        print(f"HW exec time: {res.exec_time_ns} ns")

    PLOC = pr["PLOC"]
    outs = []
    for c in range(NCORES):
        vals = res.results[c]["out"].reshape(-1)
        outs.append(vals[:pr["plocs"][c]])
    return np.concatenate(outs).astype(np.float32)
